# revision 15
# baseline (speedup 1.0000x reference)
"""Chamfer loss kernel for Trainium2 (8 NeuronCores, one batch per core).

Problem: B=8, N=M=8192, D=64 fp32.
  rd = pairwise euclidean distances x[b] vs y[b]   [B, N, M]
  loss = mean_b( sum_n min_m rd + sum_m min_n rd ) / M

Device strategy (per core = one batch):
  - sqrt is monotonic -> only need minima of SQUARED distances; sqrt+sums
    happen on host over 2*8192 values per batch.
  - d2 = x2 + y2 - 2*x.y is produced entirely by ONE bf16 matmul with an
    augmented contraction dim:
       lhsT rows (x side, [68, N]): [x_d (64) ; 1 ; 1 ; x2_hi ; x2_lo]
       rhs  rows (y side, [68, M]): [-2*y_d (64) ; y2_hi ; y2_lo ; 1 ; 1]
    so psum = sum_d x_d*(-2 y_d) + y2_hi + y2_lo + x2_hi + x2_lo = d2.
    (hi/lo bf16 splits keep the squared-norm terms at ~fp24 precision.)
  - ScalarE copies each PSUM group to one n-wide bf16 SBUF tile; VectorE
    (the bottleneck engine, bf16 tensor_tensor min at 2 elem/cycle/lane)
    then does per n-tile: ONE wide col-min accumulate into a [128, M]
    accumulator (n folded mod 128) + a fold-tree of wide TT-mins and one
    small reduce for the row mins.
  - The col accumulator is finished by PE transposes + wide DVE reduces.
Host does the final sqrt / sums / mean in float64.
(tensor_tensor_reduce / tensor_tensor_scan were evaluated: TTR faults this
runtime (NRT_EXEC_UNIT_UNRECOVERABLE), scan is ~2.5x slower than the tree.)
"""

import os

import numpy as np
import ml_dtypes

P = 128
N = 8192
D = 64
KAUG = D + 4  # 68
B = 8

_CACHE = {}

DEFAULT_ROW_MODE = "tsacc"


def _build_nc(n=N, mm_free=512, m_group=2048, row_mode="ttr", kaug=KAUG,
              skip_tail=False, repeat=1, col_tail="device"):
    import concourse.bass as bass
    import concourse.mybir as mybir
    import concourse.tile as tile
    from concourse import bacc
    from concourse.masks import make_identity

    fp32 = mybir.dt.float32
    bf16 = mybir.dt.bfloat16
    MIN = mybir.AluOpType.min

    nt_count = n // P          # n-tiles (output partition blocks)
    ngroups = n // m_group     # m groups per n-tile
    mm_per_g = m_group // mm_free

    # Bacc (not raw Bass): its compile pipeline lowers instructions with more
    # sync waits than the ISA's embedded slots into EventSemaphore insts.
    nc = bacc.Bacc("TRN2", target_bir_lowering=False, debug=False)
    xT = nc.dram_tensor("xT", [kaug, n], bf16, kind="ExternalInput")
    yT = nc.dram_tensor("yT", [kaug, n], bf16, kind="ExternalInput")
    out_slots = 3 if row_mode == "tsacc2" else 2
    out = nc.dram_tensor("out", [P, out_slots * nt_count], fp32,
                         kind="ExternalOutput")
    colout = None
    if col_tail == "host":
        # ship the lane-folded col accumulator; host does the 128-lane min
        colout = nc.dram_tensor("colout", [P, n], bf16, kind="ExternalOutput")

    with tile.TileContext(nc) as tc:
        with (
            tc.tile_pool(name="const", bufs=1) as cpool,
            tc.tile_pool(name="work", bufs=3) as wpool,
            tc.tile_pool(name="psum", bufs=2, space="PSUM") as ppool,
        ):
            xTs = cpool.tile([P, n], bf16)
            yTs = cpool.tile([P, n], bf16)
            colacc = cpool.tile([P, n], bf16)
            rowacc = cpool.tile([P, m_group], bf16)
            rowmin = cpool.tile([P, nt_count], fp32)
            if col_tail != "host":
                colmin = cpool.tile([P, nt_count], fp32)
                ident = cpool.tile([P, P], bf16)

            # chunked loads so early matmuls start before the full tensors land
            n_chunks = max(1, n // 2048)
            cw = n // n_chunks
            for c in range(n_chunks):
                nc.sync.dma_start(
                    xTs[:kaug, c * cw:(c + 1) * cw], xT[:, c * cw:(c + 1) * cw])
                nc.sync.dma_start(
                    yTs[:kaug, c * cw:(c + 1) * cw], yT[:, c * cw:(c + 1) * cw])
            if col_tail != "host":
                make_identity(nc, ident)

            if row_mode == "tt":
                rowacc_narrow = cpool.tile([P, mm_free], bf16)
            if row_mode == "ttr2":
                rowacc2 = cpool.tile([P, m_group], bf16)

            if row_mode == "fold2":
                # alias-free variant of "fold": ping-pong col accumulators and
                # alternate row-tree scratch tiles, in case in-place operands
                # demote the DVE from 2x_1P to 1x mode.
                colacc2 = cpool.tile([P, n], bf16)
                accs = [colacc, colacc2]
                vtile = cpool.tile([P, n // 4], bf16)
                for rep in range(repeat):
                    for nt in range(nt_count):
                        lhsT = xTs[:kaug, nt * P:(nt + 1) * P]
                        sfull = wpool.tile([P, n], bf16, tag="s",
                                           name="sfull", bufs=3)
                        for g in range(ngroups):
                            ps = ppool.tile([P, m_group], fp32,
                                            tag="ps", name="ps")
                            for k in range(mm_per_g):
                                nc.tensor.matmul(
                                    ps[:, k * mm_free:(k + 1) * mm_free],
                                    lhsT,
                                    yTs[:kaug,
                                        g * m_group + k * mm_free:
                                        g * m_group + (k + 1) * mm_free],
                                    start=True, stop=True)
                            nc.scalar.copy(
                                out=sfull[:, g * m_group:(g + 1) * m_group],
                                in_=ps)
                        i = (rep * nt_count + nt) % 2
                        if nt == 0 and rep == 0:
                            nc.vector.tensor_copy(out=accs[i], in_=sfull)
                        else:
                            nc.vector.tensor_tensor(
                                out=accs[i], in0=accs[1 - i], in1=sfull,
                                op=MIN)
                        # row fold tree, alternating scratch tiles (no alias)
                        u = wpool.tile([P, n // 2], bf16, tag="u",
                                       name="u", bufs=3)
                        nc.vector.tensor_tensor(
                            out=u, in0=sfull[:, :n // 2],
                            in1=sfull[:, n // 2:], op=MIN)
                        nc.vector.tensor_tensor(
                            out=vtile, in0=u[:, :n // 4],
                            in1=u[:, n // 4:], op=MIN)
                        nc.vector.tensor_tensor(
                            out=u[:, :n // 8], in0=vtile[:, :n // 8],
                            in1=vtile[:, n // 8:], op=MIN)
                        nc.vector.tensor_tensor(
                            out=vtile[:, :n // 16], in0=u[:, :n // 16],
                            in1=u[:, n // 16:n // 8], op=MIN)
                        nc.vector.tensor_reduce(
                            out=rowmin[:, nt:nt + 1], in_=vtile[:, :n // 16],
                            axis=mybir.AxisListType.X, op=MIN)
                final_colacc = accs[(repeat * nt_count - 1) % 2]
            else:
                final_colacc = colacc

            if row_mode == "tsacc2":
                # tsacc + ACT/DVE drain split: every drain_period-th n-tile,
                # the last m-group is drained from PSUM by a DVE
                # tensor_scalar (1x from PSUM, converts to bf16 into sfull
                # AND min-accumulates that group's row partial into rowmin2)
                # instead of the busier ACT engine; host takes
                # min(rowmin, rowmin2).
                drain_period = int(os.environ.get("CHAMFER_DRAIN_PERIOD",
                                                  "4"))
                drain_g = int(os.environ.get("CHAMFER_DRAIN_G", "0"))
                rowmin2 = cpool.tile([P, nt_count], fp32)
                nc.vector.memset(rowmin2, 3.0e38)
                for rep in range(repeat):
                    for nt in range(nt_count):
                        drain = (nt % drain_period == drain_period - 1)
                        lhsT = xTs[:kaug, nt * P:(nt + 1) * P]
                        sfull = wpool.tile([P, n], bf16, tag="s",
                                           name="sfull", bufs=3)
                        for g in range(ngroups):
                            ps = ppool.tile([P, m_group], fp32,
                                            tag="ps", name="ps")
                            for k in range(mm_per_g):
                                nc.tensor.matmul(
                                    ps[:, k * mm_free:(k + 1) * mm_free],
                                    lhsT,
                                    yTs[:kaug,
                                        g * m_group + k * mm_free:
                                        g * m_group + (k + 1) * mm_free],
                                    start=True, stop=True)
                            if drain and g == drain_g:
                                nc.vector.tensor_scalar(
                                    out=sfull[:, g * m_group:
                                              (g + 1) * m_group],
                                    in0=ps, scalar1=3.0e38, scalar2=None,
                                    op0=MIN, op1=MIN,
                                    accum_out=rowmin2[:, nt:nt + 1])
                            else:
                                nc.scalar.copy(
                                    out=sfull[:, g * m_group:
                                              (g + 1) * m_group],
                                    in_=ps)
                        # main row-TS covers the ACT-copied groups, which
                        # are contiguous only for drain_g == 0 or last
                        ts_lo = (m_group if (drain and drain_g == 0) else 0)
                        ts_hi = n - (m_group
                                     if (drain and drain_g == ngroups - 1)
                                     else 0)
                        if nt == 0 and rep == 0:
                            nc.vector.tensor_scalar(
                                out=colacc, in0=sfull, scalar1=3.0e38,
                                scalar2=None, op0=MIN, op1=MIN,
                                accum_out=rowmin[:, nt:nt + 1])
                        else:
                            nc.vector.tensor_tensor(
                                out=colacc, in0=colacc, in1=sfull, op=MIN)
                            scr = wpool.tile([P, n], bf16, tag="scr",
                                             name="scr", bufs=2)
                            nc.vector.tensor_scalar(
                                out=scr[:, ts_lo:ts_hi],
                                in0=sfull[:, ts_lo:ts_hi],
                                scalar1=3.0e38, scalar2=None,
                                op0=MIN, op1=MIN,
                                accum_out=rowmin[:, nt:nt + 1])

            if row_mode == "tsacc":
                # One n-wide s tile per n-tile: ONE wide col-min TT (2x) and
                # row mins via ONE tensor_scalar with min-accumulate (4x:
                # TensorScalarPtr supports 4x_2p; out=min(s,3e38)=s is a
                # throwaway wide write, accum_out = min over the free dim).
                for rep in range(repeat):
                    for nt in range(nt_count):
                        lhsT = xTs[:kaug, nt * P:(nt + 1) * P]
                        sfull = wpool.tile([P, n], bf16, tag="s",
                                           name="sfull", bufs=3)
                        for g in range(ngroups):
                            ps = ppool.tile([P, m_group], fp32,
                                            tag="ps", name="ps")
                            for k in range(mm_per_g):
                                nc.tensor.matmul(
                                    ps[:, k * mm_free:(k + 1) * mm_free],
                                    lhsT,
                                    yTs[:kaug,
                                        g * m_group + k * mm_free:
                                        g * m_group + (k + 1) * mm_free],
                                    start=True, stop=True)
                            nc.scalar.copy(
                                out=sfull[:, g * m_group:(g + 1) * m_group],
                                in_=ps)
                        if nt == 0 and rep == 0:
                            # TS both initializes colacc (out = min(s, 3e38)
                            # = s) and produces this tile's row min.
                            nc.vector.tensor_scalar(
                                out=colacc, in0=sfull, scalar1=3.0e38,
                                scalar2=None, op0=MIN, op1=MIN,
                                accum_out=rowmin[:, nt:nt + 1])
                        else:
                            nc.vector.tensor_tensor(
                                out=colacc, in0=colacc, in1=sfull, op=MIN)
                            scr = wpool.tile([P, n], bf16, tag="scr",
                                             name="scr", bufs=2)
                            nc.vector.tensor_scalar(
                                out=scr, in0=sfull, scalar1=3.0e38,
                                scalar2=None, op0=MIN, op1=MIN,
                                accum_out=rowmin[:, nt:nt + 1])

            if row_mode == "fold":
                # One n-wide s tile per n-tile: ONE wide col-min TT, and row
                # mins via a fold tree of wide TT-mins + one small reduce.
                for rep in range(repeat):
                    for nt in range(nt_count):
                        lhsT = xTs[:kaug, nt * P:(nt + 1) * P]
                        sfull = wpool.tile([P, n], bf16, tag="s",
                                           name="sfull", bufs=3)
                        for g in range(ngroups):
                            ps = ppool.tile([P, m_group], fp32,
                                            tag="ps", name="ps")
                            for k in range(mm_per_g):
                                nc.tensor.matmul(
                                    ps[:, k * mm_free:(k + 1) * mm_free],
                                    lhsT,
                                    yTs[:kaug,
                                        g * m_group + k * mm_free:
                                        g * m_group + (k + 1) * mm_free],
                                    start=True, stop=True)
                            nc.scalar.copy(
                                out=sfull[:, g * m_group:(g + 1) * m_group],
                                in_=ps)
                        if nt == 0 and rep == 0:
                            nc.vector.tensor_copy(out=colacc, in_=sfull)
                        else:
                            nc.vector.tensor_tensor(
                                out=colacc, in0=colacc, in1=sfull, op=MIN)
                        # row fold tree
                        u = wpool.tile([P, n // 2], bf16, tag="u",
                                       name="u", bufs=3)
                        nc.vector.tensor_tensor(
                            out=u, in0=sfull[:, :n // 2],
                            in1=sfull[:, n // 2:], op=MIN)
                        w = n // 2
                        while w > 512:
                            nc.vector.tensor_tensor(
                                out=u[:, :w // 2], in0=u[:, :w // 2],
                                in1=u[:, w // 2:w], op=MIN)
                            w //= 2
                        nc.vector.tensor_reduce(
                            out=rowmin[:, nt:nt + 1], in_=u[:, :w],
                            axis=mybir.AxisListType.X, op=MIN)

            for rep in range(
                    repeat if row_mode not in ("fold", "fold2", "tsacc",
                                               "tsacc2")
                    else 0):
              for nt in range(nt_count):
                lhsT = xTs[:kaug, nt * P:(nt + 1) * P]
                for g in range(ngroups):
                    ps = ppool.tile([P, m_group], fp32, tag="ps", name="ps")
                    for k in range(mm_per_g):
                        nc.tensor.matmul(
                            ps[:, k * mm_free:(k + 1) * mm_free],
                            lhsT,
                            yTs[:kaug, g * m_group + k * mm_free:
                                g * m_group + (k + 1) * mm_free],
                            start=True,
                            stop=True,
                        )
                    s = wpool.tile([P, m_group], bf16, name="s")
                    nc.scalar.copy(out=s, in_=ps)

                    # column-min accumulator (n folded into the 128 lanes)
                    csl = colacc[:, g * m_group:(g + 1) * m_group]
                    if nt == 0:
                        nc.vector.tensor_copy(out=csl, in_=s)
                    else:
                        nc.vector.tensor_tensor(out=csl, in0=csl, in1=s, op=MIN)

                    # row mins
                    if row_mode == "ttr2":
                        # like "ttr" but ping-pongs the elementwise-min
                        # accumulator to avoid in-place out/in1 aliasing
                        accs = [rowacc, rowacc2]
                        dst = accs[g % 2]
                        src = s if g == 0 else accs[1 - g % 2]
                        nc.vector.tensor_tensor_reduce(
                            out=dst,
                            in0=s,
                            in1=src,
                            scale=1.0,
                            scalar=3.0e38,
                            op0=MIN,
                            op1=MIN,
                            accum_out=rowmin[:, nt:nt + 1],
                        )
                    elif row_mode == "ttr":
                        # rowacc = min(rowacc, s) elementwise; accum_out gets
                        # min over the free dim of the updated rowacc. The
                        # last group's accum covers all m -> true row min.
                        nc.vector.tensor_tensor_reduce(
                            out=rowacc,
                            in0=s,
                            in1=(s if g == 0 else rowacc),
                            scale=1.0,
                            scalar=3.0e38,
                            op0=MIN,
                            op1=MIN,
                            accum_out=rowmin[:, nt:nt + 1],
                        )
                    else:
                        for k in range(mm_per_g):
                            ssl = s[:, k * mm_free:(k + 1) * mm_free]
                            if g == 0 and k == 0:
                                nc.vector.tensor_copy(out=rowacc_narrow, in_=ssl)
                            else:
                                nc.vector.tensor_tensor(
                                    out=rowacc_narrow, in0=rowacc_narrow,
                                    in1=ssl, op=MIN)
                        if g == ngroups - 1:
                            nc.vector.tensor_reduce(
                                out=rowmin[:, nt:nt + 1], in_=rowacc_narrow,
                                axis=mybir.AxisListType.X, op=MIN)

            # column-min finish: transpose each [128, 128] block of colacc on
            # PE, then min-reduce the (former partition) lanes on DVE.
            if col_tail == "host":
                nc.sync.dma_start(colout[:, :], final_colacc[:, :])
            elif not skip_tail:
                # batch transposes into wide bf16 PSUM tiles so the lane-min
                # runs as a few wide DVE reduces instead of nt_count small ones
                tpb = max(1, min(nt_count, (m_group * 2) // P))
                for t0 in range(0, nt_count, tpb):
                    cnt = min(tpb, nt_count - t0)
                    pt = ppool.tile([P, tpb, P], bf16, tag="ps", name="pt")
                    for i in range(cnt):
                        t = t0 + i
                        nc.tensor.transpose(
                            pt[:, i, :], final_colacc[:, t * P:(t + 1) * P], ident)
                    nc.vector.tensor_reduce(
                        out=colmin[:, t0:t0 + cnt], in_=pt[:, :cnt, :],
                        axis=mybir.AxisListType.X, op=MIN)
            else:
                nc.vector.tensor_copy(out=colmin, in_=rowmin)

            nc.sync.dma_start(out[:, :nt_count], rowmin[:, :])
            if row_mode == "tsacc2":
                nc.sync.dma_start(
                    out[:, nt_count:2 * nt_count], rowmin2[:, :])
            if col_tail != "host":
                nc.sync.dma_start(
                    out[:, (out_slots - 1) * nt_count:], colmin[:, :])

    nc.finalize()  # runs the Bacc compile passes (event sems, reg alloc, ...)
    return nc


def _prep_inputs(x, y, kaug=KAUG):
    """Build the augmented, transposed bf16 operands for each batch."""
    bf = ml_dtypes.bfloat16
    in_maps = []
    for b in range(x.shape[0]):
        xb = np.asarray(x[b], dtype=np.float32)
        yb = np.asarray(y[b], dtype=np.float32)
        n = xb.shape[0]
        x2 = np.sum(xb * xb, axis=-1)
        y2 = np.sum(yb * yb, axis=-1)
        x2_hi = x2.astype(bf)
        x2_lo = (x2 - x2_hi.astype(np.float32)).astype(bf)
        y2_hi = y2.astype(bf)
        y2_lo = (y2 - y2_hi.astype(np.float32)).astype(bf)
        ones = np.ones((1, n), dtype=bf)
        xT = np.concatenate(
            [xb.T.astype(bf), ones, ones, x2_hi[None], x2_lo[None]], axis=0)
        yT = np.concatenate(
            [(-2.0 * yb).T.astype(bf), y2_hi[None], y2_lo[None], ones, ones],
            axis=0)
        if kaug > KAUG:
            pad = np.zeros((kaug - KAUG, n), dtype=bf)
            xT = np.concatenate([xT, pad], axis=0)
            yT = np.concatenate([yT, pad], axis=0)
        in_maps.append({
            "xT": np.ascontiguousarray(xT),
            "yT": np.ascontiguousarray(yT),
        })
    return in_maps


def _postprocess(results, n=N):
    nt_count = n // P
    total = 0.0
    nb = len(results)
    for b in range(nb):
        o = np.asarray(results[b]["out"], dtype=np.float64)
        rowmin = o[:, :nt_count].T.reshape(-1)   # [n], index t*128+p
        if o.shape[1] == 3 * nt_count:
            # tsacc2: second slot holds DVE-drained group row partials
            rowmin = np.minimum(
                rowmin, o[:, nt_count:2 * nt_count].T.reshape(-1))
        if "colout" in results[b]:
            co = np.asarray(results[b]["colout"], dtype=np.float32)
            colmin = co.min(axis=0).astype(np.float64)
        else:
            colmin = o[:, -nt_count:].T.reshape(-1)
        total += np.sqrt(np.maximum(rowmin, 0.0)).sum()
        total += np.sqrt(np.maximum(colmin, 0.0)).sum()
    loss = total / nb / n
    return np.asarray(loss, dtype=np.float32)


def _get_runner(n_cores=B):
    """Build the Bass module once and return a reusable jitted runner.

    Modeled on concourse.bass2jax.run_bass_via_pjrt's multi-core branch, but
    keeps the jitted callable so repeated invocations don't re-lower."""
    key = ("runner", n_cores)
    if key in _CACHE:
        return _CACHE[key]

    import jax
    from jax.experimental.shard_map import shard_map
    from jax.sharding import Mesh, PartitionSpec
    from concourse import bass2jax, mybir

    nc = _build_nc(row_mode=os.environ.get("CHAMFER_ROW_MODE",
                                           DEFAULT_ROW_MODE),
                   col_tail=os.environ.get("CHAMFER_COL_TAIL", "device"))

    bass2jax.install_neuronx_cc_hook()
    assert nc.dbg_addr is None

    partition_name = (
        nc.partition_id_tensor.name if nc.partition_id_tensor else None)
    in_names, out_names, out_avals = [], [], []
    for alloc in nc.m.functions[0].allocations:
        if not isinstance(alloc, mybir.MemoryLocationSet):
            continue
        name = alloc.memorylocations[0].name
        if alloc.kind == "ExternalInput":
            if name != partition_name:
                in_names.append(name)
        elif alloc.kind == "ExternalOutput":
            out_names.append(name)
            out_avals.append(jax.core.ShapedArray(
                tuple(alloc.tensor_shape), mybir.dt.np(alloc.dtype)))
    n_params = len(in_names)
    n_outs = len(out_avals)
    all_in_names = list(in_names) + list(out_names)
    if partition_name is not None:
        all_in_names.append(partition_name)
    donate = tuple(range(n_params, n_params + n_outs))

    def _body(*args):
        operands = list(args)
        if partition_name is not None:
            operands.append(bass2jax.partition_id_tensor())
        outs = bass2jax._bass_exec_p.bind(
            *operands,
            out_avals=tuple(out_avals),
            in_names=tuple(all_in_names),
            out_names=tuple(out_names),
            lowering_input_output_aliases=(),
            sim_require_finite=True,
            sim_require_nnan=True,
            nc=nc,
        )
        return tuple(outs)

    devices = jax.devices()[:n_cores]
    mesh = Mesh(np.asarray(devices), ("core",))
    sharded = jax.jit(
        shard_map(
            _body, mesh=mesh,
            in_specs=(PartitionSpec("core"),) * (n_params + n_outs),
            out_specs=(PartitionSpec("core"),) * n_outs,
            check_rep=False,
        ),
        donate_argnums=donate,
        keep_unused=True,
    )

    def run(in_maps):
        per_core = [[np.asarray(m[nm]) for nm in in_names] for m in in_maps]
        concat_in = [
            np.concatenate([per_core[c][i] for c in range(n_cores)], axis=0)
            for i in range(n_params)
        ]
        concat_zeros = [
            np.zeros((n_cores * a.shape[0], *a.shape[1:]), a.dtype)
            for a in out_avals
        ]
        out_arrs = sharded(*concat_in, *concat_zeros)
        jax.block_until_ready(out_arrs)
        return [
            {nm: np.asarray(out_arrs[i]).reshape(
                n_cores, *out_avals[i].shape)[c]
             for i, nm in enumerate(out_names)}
            for c in range(n_cores)
        ]

    _CACHE[key] = run
    return run


def kernel(x, y):
    import time

    x = np.asarray(x)
    y = np.asarray(y)
    in_maps = _prep_inputs(x, y)
    run = _get_runner(n_cores=len(in_maps))
    # the device occasionally wedges transiently on a fresh NEFF's first
    # execution (NRT_EXEC_UNIT_UNRECOVERABLE); a retry reliably clears it
    last_err = None
    for attempt in range(4):
        try:
            results = run(in_maps)
            return _postprocess(results)
        except Exception as e:  # noqa: BLE001 - retry any runtime failure
            last_err = e
            time.sleep(2.0)
            try:
                import jax
                jax.clear_caches()
            except Exception:
                pass
            _CACHE.clear()  # rebuild runner; NEFF recompile is disk-cached
            run = _get_runner(n_cores=len(in_maps))
    raise last_err



# revision 20
# speedup vs baseline: 1.4586x; 1.4586x over previous
"""Chamfer loss kernel for Trainium2 (8 NeuronCores, one batch per core).

Problem: B=8, N=M=8192, D=64 fp32.
  rd = pairwise euclidean distances x[b] vs y[b]   [B, N, M]
  loss = mean_b( sum_n min_m rd + sum_m min_n rd ) / M

Device strategy (per core = one batch):
  - sqrt is monotonic -> only need minima of SQUARED distances; sqrt+sums
    happen on host over 2*8192 values per batch.
  - d2 = x2 + y2 - 2*x.y is produced entirely by ONE bf16 matmul with an
    augmented contraction dim:
       lhsT rows (x side, [68, N]): [x_d (64) ; 1 ; 1 ; x2_hi ; x2_lo]
       rhs  rows (y side, [68, M]): [-2*y_d (64) ; y2_hi ; y2_lo ; 1 ; 1]
    so psum = sum_d x_d*(-2 y_d) + y2_hi + y2_lo + x2_hi + x2_lo = d2.
    (hi/lo bf16 splits keep the squared-norm terms at ~fp24 precision.)
  - ScalarE copies each PSUM group to one n-wide bf16 SBUF tile; VectorE
    (the bottleneck engine, bf16 tensor_tensor min at 2 elem/cycle/lane)
    then does per n-tile: ONE wide col-min accumulate into a [128, M]
    accumulator (n folded mod 128) + a fold-tree of wide TT-mins and one
    small reduce for the row mins.
  - The col accumulator is finished by PE transposes + wide DVE reduces.
Host does the final sqrt / sums / mean in float64.
(tensor_tensor_reduce / tensor_tensor_scan were evaluated: TTR faults this
runtime (NRT_EXEC_UNIT_UNRECOVERABLE), scan is ~2.5x slower than the tree.)

Alternatives evaluated on HW (all lose to "fold"; measured with interleaved
129-long chained-submission benches, bench.py):
  - tsacc: row min via ONE tensor_scalar(op0=min, op1=min, accum_out):
    the cost model claims 4x_2p for TensorScalarPtr+accum, but ON HW the
    accumulate path runs at ~1x (7.8-10.2us per [128,8192] op vs 2.7-3.0us
    without accum) -> ~25% slower than fold overall.  The plain-TS 4x is
    real; the accumulator kills it.
  - tsacc2: tsacc + rotating DVE-drain of one PSUM group: worse still
    (scheduling bubbles).
  - foldx@dd: draining some PSUM groups via DVE tensor_copy (micro-bench
    says ~2us/group vs ACT ~3us) is a wash at dd=4 and worse at dd=2.
  - gpsimd (Pool) TensorTensor: rejected by walrus codegen
    ("Instruction engine check failed (Pool)") - no Pool offload possible.
  - gpsimd-initiated DMA with accum_op=min (SBUF->SBUF): rejected by the
    BIR verifier (visitInstDMACopy throws).
  - DMA cast fp32->bf16 from PSUM: dma_start asserts SBUF/DRAM only.
  - matmul writing bf16 PSUM: asserts "matmul output must be fp32".
  - mm_free=1024: invalid ISA (PSUM bank = 512 fp32 is the matmul max).
  - tree stop-width 1024/2048, fold2 (alias-free), m_group variants: noise.
Engine-time structure (TimelineSim, per core): PE matmuls 227us, ACT PSUM
drains ~485us, DVE col-TT+row-tree ~593us; HW ~740-820us.  The hard wall:
only ACT/DVE can read PSUM (1 elem/cyc effective for ACT, fp32 1x for DVE),
TT min is capped at 2x, and every single-src 4x op (tensor_copy/plain TS)
either cannot reduce or loses its speed with accum_out.
"""

import os

import numpy as np
import ml_dtypes

P = 128
N = 8192
D = 64
KAUG = D + 4  # 68
B = 8

_CACHE = {}

DEFAULT_ROW_MODE = "fold"


def _build_nc(n=N, mm_free=512, m_group=2048, row_mode="ttr", kaug=KAUG,
              skip_tail=False, repeat=1, col_tail="device"):
    import concourse.bass as bass
    import concourse.mybir as mybir
    import concourse.tile as tile
    from concourse import bacc
    from concourse.masks import make_identity

    fp32 = mybir.dt.float32
    bf16 = mybir.dt.bfloat16
    MIN = mybir.AluOpType.min

    nt_count = n // P          # n-tiles (output partition blocks)
    ngroups = n // m_group     # m groups per n-tile
    mm_per_g = m_group // mm_free

    # Bacc (not raw Bass): its compile pipeline lowers instructions with more
    # sync waits than the ISA's embedded slots into EventSemaphore insts.
    nc = bacc.Bacc("TRN2", target_bir_lowering=False, debug=False)
    xT = nc.dram_tensor("xT", [kaug, n], bf16, kind="ExternalInput")
    yT = nc.dram_tensor("yT", [kaug, n], bf16, kind="ExternalInput")
    out_slots = 3 if row_mode == "tsacc2" else 2
    out = nc.dram_tensor("out", [P, out_slots * nt_count], fp32,
                         kind="ExternalOutput")
    colout = None
    if col_tail == "host":
        # ship the lane-folded col accumulator; host does the 128-lane min
        colout = nc.dram_tensor("colout", [P, n], bf16, kind="ExternalOutput")

    with tile.TileContext(nc) as tc:
        with (
            tc.tile_pool(name="const", bufs=1) as cpool,
            tc.tile_pool(name="work", bufs=3) as wpool,
            tc.tile_pool(name="psum", bufs=2, space="PSUM") as ppool,
        ):
            xTs = cpool.tile([P, n], bf16)
            yTs = cpool.tile([P, n], bf16)
            colacc = cpool.tile([P, n], bf16)
            rowacc = cpool.tile([P, m_group], bf16)
            rowmin = cpool.tile([P, nt_count], fp32)
            if col_tail != "host":
                colmin = cpool.tile([P, nt_count], fp32)
                ident = cpool.tile([P, P], bf16)

            # chunked loads so early matmuls start before the full tensors land
            n_chunks = max(1, n // 2048)
            cw = n // n_chunks
            for c in range(n_chunks):
                nc.sync.dma_start(
                    xTs[:kaug, c * cw:(c + 1) * cw], xT[:, c * cw:(c + 1) * cw])
                nc.sync.dma_start(
                    yTs[:kaug, c * cw:(c + 1) * cw], yT[:, c * cw:(c + 1) * cw])
            if col_tail != "host":
                make_identity(nc, ident)

            if row_mode == "tt":
                rowacc_narrow = cpool.tile([P, mm_free], bf16)
            if row_mode == "ttr2":
                rowacc2 = cpool.tile([P, m_group], bf16)

            if row_mode == "fold2":
                # alias-free variant of "fold": ping-pong col accumulators and
                # alternate row-tree scratch tiles, in case in-place operands
                # demote the DVE from 2x_1P to 1x mode.
                colacc2 = cpool.tile([P, n], bf16)
                accs = [colacc, colacc2]
                vtile = cpool.tile([P, n // 4], bf16)
                for rep in range(repeat):
                    for nt in range(nt_count):
                        lhsT = xTs[:kaug, nt * P:(nt + 1) * P]
                        sfull = wpool.tile([P, n], bf16, tag="s",
                                           name="sfull", bufs=3)
                        for g in range(ngroups):
                            ps = ppool.tile([P, m_group], fp32,
                                            tag="ps", name="ps")
                            for k in range(mm_per_g):
                                nc.tensor.matmul(
                                    ps[:, k * mm_free:(k + 1) * mm_free],
                                    lhsT,
                                    yTs[:kaug,
                                        g * m_group + k * mm_free:
                                        g * m_group + (k + 1) * mm_free],
                                    start=True, stop=True)
                            nc.scalar.copy(
                                out=sfull[:, g * m_group:(g + 1) * m_group],
                                in_=ps)
                        i = (rep * nt_count + nt) % 2
                        if nt == 0 and rep == 0:
                            nc.vector.tensor_copy(out=accs[i], in_=sfull)
                        else:
                            nc.vector.tensor_tensor(
                                out=accs[i], in0=accs[1 - i], in1=sfull,
                                op=MIN)
                        # row fold tree, alternating scratch tiles (no alias)
                        u = wpool.tile([P, n // 2], bf16, tag="u",
                                       name="u", bufs=3)
                        nc.vector.tensor_tensor(
                            out=u, in0=sfull[:, :n // 2],
                            in1=sfull[:, n // 2:], op=MIN)
                        nc.vector.tensor_tensor(
                            out=vtile, in0=u[:, :n // 4],
                            in1=u[:, n // 4:], op=MIN)
                        nc.vector.tensor_tensor(
                            out=u[:, :n // 8], in0=vtile[:, :n // 8],
                            in1=vtile[:, n // 8:], op=MIN)
                        nc.vector.tensor_tensor(
                            out=vtile[:, :n // 16], in0=u[:, :n // 16],
                            in1=u[:, n // 16:n // 8], op=MIN)
                        nc.vector.tensor_reduce(
                            out=rowmin[:, nt:nt + 1], in_=vtile[:, :n // 16],
                            axis=mybir.AxisListType.X, op=MIN)
                final_colacc = accs[(repeat * nt_count - 1) % 2]
            else:
                final_colacc = colacc

            if row_mode == "tsacc2":
                # tsacc + ACT/DVE drain split: every drain_period-th n-tile,
                # the last m-group is drained from PSUM by a DVE
                # tensor_scalar (1x from PSUM, converts to bf16 into sfull
                # AND min-accumulates that group's row partial into rowmin2)
                # instead of the busier ACT engine; host takes
                # min(rowmin, rowmin2).
                drain_period = int(os.environ.get("CHAMFER_DRAIN_PERIOD",
                                                  "4"))
                drain_g = int(os.environ.get("CHAMFER_DRAIN_G", "0"))
                rowmin2 = cpool.tile([P, nt_count], fp32)
                nc.vector.memset(rowmin2, 3.0e38)
                for rep in range(repeat):
                    for nt in range(nt_count):
                        drain = (nt % drain_period == drain_period - 1)
                        lhsT = xTs[:kaug, nt * P:(nt + 1) * P]
                        sfull = wpool.tile([P, n], bf16, tag="s",
                                           name="sfull", bufs=3)
                        for g in range(ngroups):
                            ps = ppool.tile([P, m_group], fp32,
                                            tag="ps", name="ps")
                            for k in range(mm_per_g):
                                nc.tensor.matmul(
                                    ps[:, k * mm_free:(k + 1) * mm_free],
                                    lhsT,
                                    yTs[:kaug,
                                        g * m_group + k * mm_free:
                                        g * m_group + (k + 1) * mm_free],
                                    start=True, stop=True)
                            if drain and g == drain_g:
                                nc.vector.tensor_scalar(
                                    out=sfull[:, g * m_group:
                                              (g + 1) * m_group],
                                    in0=ps, scalar1=3.0e38, scalar2=None,
                                    op0=MIN, op1=MIN,
                                    accum_out=rowmin2[:, nt:nt + 1])
                            else:
                                nc.scalar.copy(
                                    out=sfull[:, g * m_group:
                                              (g + 1) * m_group],
                                    in_=ps)
                        # main row-TS covers the ACT-copied groups, which
                        # are contiguous only for drain_g == 0 or last
                        ts_lo = (m_group if (drain and drain_g == 0) else 0)
                        ts_hi = n - (m_group
                                     if (drain and drain_g == ngroups - 1)
                                     else 0)
                        if nt == 0 and rep == 0:
                            nc.vector.tensor_scalar(
                                out=colacc, in0=sfull, scalar1=3.0e38,
                                scalar2=None, op0=MIN, op1=MIN,
                                accum_out=rowmin[:, nt:nt + 1])
                        else:
                            nc.vector.tensor_tensor(
                                out=colacc, in0=colacc, in1=sfull, op=MIN)
                            scr = wpool.tile([P, n], bf16, tag="scr",
                                             name="scr", bufs=2)
                            nc.vector.tensor_scalar(
                                out=scr[:, ts_lo:ts_hi],
                                in0=sfull[:, ts_lo:ts_hi],
                                scalar1=3.0e38, scalar2=None,
                                op0=MIN, op1=MIN,
                                accum_out=rowmin[:, nt:nt + 1])

            if row_mode == "foldx":
                # "fold" with HW-measured rebalancing knobs:
                #   CHAMFER_DVEDRAIN_PERIOD=k: every k-th n-tile, group 0 is
                #     drained PSUM->SBUF by DVE tensor_copy (HW: ~2us/group,
                #     faster than the erratum-slowed ACT ~3us) instead of ACT.
                #   CHAMFER_POOL_COLG=1: gpsimd does the col-min TT for the
                #     last m-group (frees ~1.1us/tile of DVE).
                #   CHAMFER_POOL_TREE=1: gpsimd does the 1024- and 512-wide
                #     row-tree levels (frees ~0.9us/tile of DVE).
                dve_period = int(os.environ.get(
                    "CHAMFER_DVEDRAIN_PERIOD", "0"))
                pool_colg = int(os.environ.get("CHAMFER_POOL_COLG", "0"))
                pool_tree = int(os.environ.get("CHAMFER_POOL_TREE", "0"))
                for rep in range(repeat):
                    for nt in range(nt_count):
                        dve_drain = dve_period and (
                            nt % dve_period == dve_period - 1)
                        lhsT = xTs[:kaug, nt * P:(nt + 1) * P]
                        sfull = wpool.tile([P, n], bf16, tag="s",
                                           name="sfull", bufs=3)
                        for g in range(ngroups):
                            ps = ppool.tile([P, m_group], fp32,
                                            tag="ps", name="ps")
                            for k in range(mm_per_g):
                                nc.tensor.matmul(
                                    ps[:, k * mm_free:(k + 1) * mm_free],
                                    lhsT,
                                    yTs[:kaug,
                                        g * m_group + k * mm_free:
                                        g * m_group + (k + 1) * mm_free],
                                    start=True, stop=True)
                            gsl = sfull[:, g * m_group:(g + 1) * m_group]
                            if dve_drain and g == 0:
                                nc.vector.tensor_copy(out=gsl, in_=ps)
                            else:
                                nc.scalar.copy(out=gsl, in_=ps)
                        # col-min accumulate
                        if nt == 0 and rep == 0:
                            nc.vector.tensor_copy(out=colacc, in_=sfull)
                        elif pool_colg:
                            csp = n - m_group
                            nc.vector.tensor_tensor(
                                out=colacc[:, :csp], in0=colacc[:, :csp],
                                in1=sfull[:, :csp], op=MIN)
                            nc.gpsimd.tensor_tensor(
                                out=colacc[:, csp:], in0=colacc[:, csp:],
                                in1=sfull[:, csp:], op=MIN)
                        else:
                            nc.vector.tensor_tensor(
                                out=colacc, in0=colacc, in1=sfull, op=MIN)
                        # row fold tree
                        u = wpool.tile([P, n // 2], bf16, tag="u",
                                       name="u", bufs=3)
                        nc.vector.tensor_tensor(
                            out=u, in0=sfull[:, :n // 2],
                            in1=sfull[:, n // 2:], op=MIN)
                        nc.vector.tensor_tensor(
                            out=u[:, :n // 4], in0=u[:, :n // 4],
                            in1=u[:, n // 4:n // 2], op=MIN)
                        w = n // 4
                        while w > 512:
                            eng = nc.gpsimd if pool_tree else nc.vector
                            eng.tensor_tensor(
                                out=u[:, :w // 2], in0=u[:, :w // 2],
                                in1=u[:, w // 2:w], op=MIN)
                            w //= 2
                        nc.vector.tensor_reduce(
                            out=rowmin[:, nt:nt + 1], in_=u[:, :w],
                            axis=mybir.AxisListType.X, op=MIN)

            if row_mode == "tsacc":
                # One n-wide s tile per n-tile: ONE wide col-min TT (2x) and
                # row mins via ONE tensor_scalar with min-accumulate (4x:
                # TensorScalarPtr supports 4x_2p; out=min(s,3e38)=s is a
                # throwaway wide write, accum_out = min over the free dim).
                for rep in range(repeat):
                    for nt in range(nt_count):
                        lhsT = xTs[:kaug, nt * P:(nt + 1) * P]
                        sfull = wpool.tile([P, n], bf16, tag="s",
                                           name="sfull", bufs=3)
                        for g in range(ngroups):
                            ps = ppool.tile([P, m_group], fp32,
                                            tag="ps", name="ps")
                            for k in range(mm_per_g):
                                nc.tensor.matmul(
                                    ps[:, k * mm_free:(k + 1) * mm_free],
                                    lhsT,
                                    yTs[:kaug,
                                        g * m_group + k * mm_free:
                                        g * m_group + (k + 1) * mm_free],
                                    start=True, stop=True)
                            nc.scalar.copy(
                                out=sfull[:, g * m_group:(g + 1) * m_group],
                                in_=ps)
                        if nt == 0 and rep == 0:
                            # TS both initializes colacc (out = min(s, 3e38)
                            # = s) and produces this tile's row min.
                            nc.vector.tensor_scalar(
                                out=colacc, in0=sfull, scalar1=3.0e38,
                                scalar2=None, op0=MIN, op1=MIN,
                                accum_out=rowmin[:, nt:nt + 1])
                        else:
                            nc.vector.tensor_tensor(
                                out=colacc, in0=colacc, in1=sfull, op=MIN)
                            scr = wpool.tile([P, n], bf16, tag="scr",
                                             name="scr", bufs=2)
                            nc.vector.tensor_scalar(
                                out=scr, in0=sfull, scalar1=3.0e38,
                                scalar2=None, op0=MIN, op1=MIN,
                                accum_out=rowmin[:, nt:nt + 1])

            if row_mode == "fold":
                # One n-wide s tile per n-tile: ONE wide col-min TT, and row
                # mins via a fold tree of wide TT-mins + one small reduce.
                for rep in range(repeat):
                    for nt in range(nt_count):
                        lhsT = xTs[:kaug, nt * P:(nt + 1) * P]
                        sfull = wpool.tile([P, n], bf16, tag="s",
                                           name="sfull", bufs=3)
                        for g in range(ngroups):
                            ps = ppool.tile([P, m_group], fp32,
                                            tag="ps", name="ps")
                            for k in range(mm_per_g):
                                nc.tensor.matmul(
                                    ps[:, k * mm_free:(k + 1) * mm_free],
                                    lhsT,
                                    yTs[:kaug,
                                        g * m_group + k * mm_free:
                                        g * m_group + (k + 1) * mm_free],
                                    start=True, stop=True)
                            nc.scalar.copy(
                                out=sfull[:, g * m_group:(g + 1) * m_group],
                                in_=ps)
                        if nt == 0 and rep == 0:
                            nc.vector.tensor_copy(out=colacc, in_=sfull)
                        else:
                            nc.vector.tensor_tensor(
                                out=colacc, in0=colacc, in1=sfull, op=MIN)
                        # row fold tree
                        u = wpool.tile([P, n // 2], bf16, tag="u",
                                       name="u", bufs=3)
                        nc.vector.tensor_tensor(
                            out=u, in0=sfull[:, :n // 2],
                            in1=sfull[:, n // 2:], op=MIN)
                        w = n // 2
                        stop_w = int(os.environ.get("CHAMFER_TREE_STOP",
                                                    "512"))
                        while w > stop_w:
                            nc.vector.tensor_tensor(
                                out=u[:, :w // 2], in0=u[:, :w // 2],
                                in1=u[:, w // 2:w], op=MIN)
                            w //= 2
                        nc.vector.tensor_reduce(
                            out=rowmin[:, nt:nt + 1], in_=u[:, :w],
                            axis=mybir.AxisListType.X, op=MIN)

            for rep in range(
                    repeat if row_mode not in ("fold", "fold2", "tsacc",
                                               "tsacc2", "foldx")
                    else 0):
              for nt in range(nt_count):
                lhsT = xTs[:kaug, nt * P:(nt + 1) * P]
                for g in range(ngroups):
                    ps = ppool.tile([P, m_group], fp32, tag="ps", name="ps")
                    for k in range(mm_per_g):
                        nc.tensor.matmul(
                            ps[:, k * mm_free:(k + 1) * mm_free],
                            lhsT,
                            yTs[:kaug, g * m_group + k * mm_free:
                                g * m_group + (k + 1) * mm_free],
                            start=True,
                            stop=True,
                        )
                    s = wpool.tile([P, m_group], bf16, name="s")
                    nc.scalar.copy(out=s, in_=ps)

                    # column-min accumulator (n folded into the 128 lanes)
                    csl = colacc[:, g * m_group:(g + 1) * m_group]
                    if nt == 0:
                        nc.vector.tensor_copy(out=csl, in_=s)
                    else:
                        nc.vector.tensor_tensor(out=csl, in0=csl, in1=s, op=MIN)

                    # row mins
                    if row_mode == "ttr2":
                        # like "ttr" but ping-pongs the elementwise-min
                        # accumulator to avoid in-place out/in1 aliasing
                        accs = [rowacc, rowacc2]
                        dst = accs[g % 2]
                        src = s if g == 0 else accs[1 - g % 2]
                        nc.vector.tensor_tensor_reduce(
                            out=dst,
                            in0=s,
                            in1=src,
                            scale=1.0,
                            scalar=3.0e38,
                            op0=MIN,
                            op1=MIN,
                            accum_out=rowmin[:, nt:nt + 1],
                        )
                    elif row_mode == "ttr":
                        # rowacc = min(rowacc, s) elementwise; accum_out gets
                        # min over the free dim of the updated rowacc. The
                        # last group's accum covers all m -> true row min.
                        nc.vector.tensor_tensor_reduce(
                            out=rowacc,
                            in0=s,
                            in1=(s if g == 0 else rowacc),
                            scale=1.0,
                            scalar=3.0e38,
                            op0=MIN,
                            op1=MIN,
                            accum_out=rowmin[:, nt:nt + 1],
                        )
                    else:
                        for k in range(mm_per_g):
                            ssl = s[:, k * mm_free:(k + 1) * mm_free]
                            if g == 0 and k == 0:
                                nc.vector.tensor_copy(out=rowacc_narrow, in_=ssl)
                            else:
                                nc.vector.tensor_tensor(
                                    out=rowacc_narrow, in0=rowacc_narrow,
                                    in1=ssl, op=MIN)
                        if g == ngroups - 1:
                            nc.vector.tensor_reduce(
                                out=rowmin[:, nt:nt + 1], in_=rowacc_narrow,
                                axis=mybir.AxisListType.X, op=MIN)

            # column-min finish: transpose each [128, 128] block of colacc on
            # PE, then min-reduce the (former partition) lanes on DVE.
            if col_tail == "host":
                nc.sync.dma_start(colout[:, :], final_colacc[:, :])
            elif not skip_tail:
                # batch transposes into wide bf16 PSUM tiles so the lane-min
                # runs as a few wide DVE reduces instead of nt_count small ones
                tpb = max(1, min(nt_count, (m_group * 2) // P))
                for t0 in range(0, nt_count, tpb):
                    cnt = min(tpb, nt_count - t0)
                    pt = ppool.tile([P, tpb, P], bf16, tag="ps", name="pt")
                    for i in range(cnt):
                        t = t0 + i
                        nc.tensor.transpose(
                            pt[:, i, :], final_colacc[:, t * P:(t + 1) * P], ident)
                    nc.vector.tensor_reduce(
                        out=colmin[:, t0:t0 + cnt], in_=pt[:, :cnt, :],
                        axis=mybir.AxisListType.X, op=MIN)
            else:
                nc.vector.tensor_copy(out=colmin, in_=rowmin)

            nc.sync.dma_start(out[:, :nt_count], rowmin[:, :])
            if row_mode == "tsacc2":
                nc.sync.dma_start(
                    out[:, nt_count:2 * nt_count], rowmin2[:, :])
            if col_tail != "host":
                nc.sync.dma_start(
                    out[:, (out_slots - 1) * nt_count:], colmin[:, :])

    nc.finalize()  # runs the Bacc compile passes (event sems, reg alloc, ...)
    return nc


def _prep_inputs(x, y, kaug=KAUG):
    """Build the augmented, transposed bf16 operands for each batch."""
    bf = ml_dtypes.bfloat16
    in_maps = []
    for b in range(x.shape[0]):
        xb = np.asarray(x[b], dtype=np.float32)
        yb = np.asarray(y[b], dtype=np.float32)
        n = xb.shape[0]
        x2 = np.sum(xb * xb, axis=-1)
        y2 = np.sum(yb * yb, axis=-1)
        x2_hi = x2.astype(bf)
        x2_lo = (x2 - x2_hi.astype(np.float32)).astype(bf)
        y2_hi = y2.astype(bf)
        y2_lo = (y2 - y2_hi.astype(np.float32)).astype(bf)
        ones = np.ones((1, n), dtype=bf)
        xT = np.concatenate(
            [xb.T.astype(bf), ones, ones, x2_hi[None], x2_lo[None]], axis=0)
        yT = np.concatenate(
            [(-2.0 * yb).T.astype(bf), y2_hi[None], y2_lo[None], ones, ones],
            axis=0)
        if kaug > KAUG:
            pad = np.zeros((kaug - KAUG, n), dtype=bf)
            xT = np.concatenate([xT, pad], axis=0)
            yT = np.concatenate([yT, pad], axis=0)
        in_maps.append({
            "xT": np.ascontiguousarray(xT),
            "yT": np.ascontiguousarray(yT),
        })
    return in_maps


def _postprocess(results, n=N):
    nt_count = n // P
    total = 0.0
    nb = len(results)
    for b in range(nb):
        o = np.asarray(results[b]["out"], dtype=np.float64)
        rowmin = o[:, :nt_count].T.reshape(-1)   # [n], index t*128+p
        if o.shape[1] == 3 * nt_count:
            # tsacc2: second slot holds DVE-drained group row partials
            rowmin = np.minimum(
                rowmin, o[:, nt_count:2 * nt_count].T.reshape(-1))
        if "colout" in results[b]:
            co = np.asarray(results[b]["colout"], dtype=np.float32)
            colmin = co.min(axis=0).astype(np.float64)
        else:
            colmin = o[:, -nt_count:].T.reshape(-1)
        total += np.sqrt(np.maximum(rowmin, 0.0)).sum()
        total += np.sqrt(np.maximum(colmin, 0.0)).sum()
    loss = total / nb / n
    return np.asarray(loss, dtype=np.float32)


def _get_runner(n_cores=B):
    """Build the Bass module once and return a reusable jitted runner.

    Modeled on concourse.bass2jax.run_bass_via_pjrt's multi-core branch, but
    keeps the jitted callable so repeated invocations don't re-lower."""
    key = ("runner", n_cores)
    if key in _CACHE:
        return _CACHE[key]

    import jax
    from jax.experimental.shard_map import shard_map
    from jax.sharding import Mesh, PartitionSpec
    from concourse import bass2jax, mybir

    nc = _build_nc(row_mode=os.environ.get("CHAMFER_ROW_MODE",
                                           DEFAULT_ROW_MODE),
                   col_tail=os.environ.get("CHAMFER_COL_TAIL", "device"))

    bass2jax.install_neuronx_cc_hook()
    assert nc.dbg_addr is None

    partition_name = (
        nc.partition_id_tensor.name if nc.partition_id_tensor else None)
    in_names, out_names, out_avals = [], [], []
    for alloc in nc.m.functions[0].allocations:
        if not isinstance(alloc, mybir.MemoryLocationSet):
            continue
        name = alloc.memorylocations[0].name
        if alloc.kind == "ExternalInput":
            if name != partition_name:
                in_names.append(name)
        elif alloc.kind == "ExternalOutput":
            out_names.append(name)
            out_avals.append(jax.core.ShapedArray(
                tuple(alloc.tensor_shape), mybir.dt.np(alloc.dtype)))
    n_params = len(in_names)
    n_outs = len(out_avals)
    all_in_names = list(in_names) + list(out_names)
    if partition_name is not None:
        all_in_names.append(partition_name)
    donate = tuple(range(n_params, n_params + n_outs))

    def _body(*args):
        operands = list(args)
        if partition_name is not None:
            operands.append(bass2jax.partition_id_tensor())
        outs = bass2jax._bass_exec_p.bind(
            *operands,
            out_avals=tuple(out_avals),
            in_names=tuple(all_in_names),
            out_names=tuple(out_names),
            lowering_input_output_aliases=(),
            sim_require_finite=True,
            sim_require_nnan=True,
            nc=nc,
        )
        return tuple(outs)

    devices = jax.devices()[:n_cores]
    mesh = Mesh(np.asarray(devices), ("core",))
    sharded = jax.jit(
        shard_map(
            _body, mesh=mesh,
            in_specs=(PartitionSpec("core"),) * (n_params + n_outs),
            out_specs=(PartitionSpec("core"),) * n_outs,
            check_rep=False,
        ),
        donate_argnums=donate,
        keep_unused=True,
    )

    def run(in_maps):
        per_core = [[np.asarray(m[nm]) for nm in in_names] for m in in_maps]
        concat_in = [
            np.concatenate([per_core[c][i] for c in range(n_cores)], axis=0)
            for i in range(n_params)
        ]
        concat_zeros = [
            np.zeros((n_cores * a.shape[0], *a.shape[1:]), a.dtype)
            for a in out_avals
        ]
        out_arrs = sharded(*concat_in, *concat_zeros)
        jax.block_until_ready(out_arrs)
        return [
            {nm: np.asarray(out_arrs[i]).reshape(
                n_cores, *out_avals[i].shape)[c]
             for i, nm in enumerate(out_names)}
            for c in range(n_cores)
        ]

    _CACHE[key] = run
    return run


def kernel(x, y):
    import time

    x = np.asarray(x)
    y = np.asarray(y)
    in_maps = _prep_inputs(x, y)
    run = _get_runner(n_cores=len(in_maps))
    # the device occasionally wedges transiently on a fresh NEFF's first
    # execution (NRT_EXEC_UNIT_UNRECOVERABLE); a retry reliably clears it
    last_err = None
    for attempt in range(4):
        try:
            results = run(in_maps)
            return _postprocess(results)
        except Exception as e:  # noqa: BLE001 - retry any runtime failure
            last_err = e
            time.sleep(2.0)
            try:
                import jax
                jax.clear_caches()
            except Exception:
                pass
            _CACHE.clear()  # rebuild runner; NEFF recompile is disk-cached
            run = _get_runner(n_cores=len(in_maps))
    raise last_err



# revision 27
# speedup vs baseline: 1.6037x; 1.0994x over previous
"""Chamfer loss kernel for Trainium2 (8 NeuronCores, one batch per core).

Problem: B=8, N=M=8192, D=64 fp32.
  rd = pairwise euclidean distances x[b] vs y[b]   [B, N, M]
  loss = mean_b( sum_n min_m rd + sum_m min_n rd ) / M

Device strategy (per core = one batch):
  - sqrt is monotonic -> only need minima of SQUARED distances; sqrt+sums
    happen on host over 2*8192 values per batch.
  - d2 = x2 + y2 - 2*x.y is produced entirely by ONE bf16 matmul with an
    augmented contraction dim:
       lhsT rows (x side, [68, N]): [x_d (64) ; 1 ; 1 ; x2_hi ; x2_lo]
       rhs  rows (y side, [68, M]): [-2*y_d (64) ; y2_hi ; y2_lo ; 1 ; 1]
    so psum = sum_d x_d*(-2 y_d) + y2_hi + y2_lo + x2_hi + x2_lo = d2.
    (hi/lo bf16 splits keep the squared-norm terms at ~fp24 precision.)
  - ScalarE copies each PSUM group to one n-wide bf16 SBUF tile; VectorE
    (the bottleneck engine, bf16 tensor_tensor min at 2 elem/cycle/lane)
    then does per n-tile: ONE wide col-min accumulate into a [128, M]
    accumulator (n folded mod 128) + a fold-tree of wide TT-mins and one
    small reduce for the row mins.
  - The col accumulator is finished by PE transposes + wide DVE reduces.
Host does the final sqrt / sums / mean in float64.
(tensor_tensor_reduce / tensor_tensor_scan were evaluated: TTR faults this
runtime (NRT_EXEC_UNIT_UNRECOVERABLE), scan is ~2.5x slower than the tree.)

Alternatives evaluated on HW (all lose to "fold"; measured with interleaved
129-long chained-submission benches, bench.py):
  - tsacc: row min via ONE tensor_scalar(op0=min, op1=min, accum_out):
    the cost model claims 4x_2p for TensorScalarPtr+accum, but ON HW the
    accumulate path runs at ~1x (7.8-10.2us per [128,8192] op vs 2.7-3.0us
    without accum) -> ~25% slower than fold overall.  The plain-TS 4x is
    real; the accumulator kills it.
  - tsacc2: tsacc + rotating DVE-drain of one PSUM group: worse still
    (scheduling bubbles).
  - foldx@dd: draining some PSUM groups via DVE tensor_copy (micro-bench
    says ~2us/group vs ACT ~3us) is a wash at dd=4 and worse at dd=2.
  - gpsimd (Pool) TensorTensor: rejected by walrus codegen
    ("Instruction engine check failed (Pool)") - no Pool offload possible.
  - gpsimd-initiated DMA with accum_op=min (SBUF->SBUF): rejected by the
    BIR verifier (visitInstDMACopy throws).
  - DMA cast fp32->bf16 from PSUM: dma_start asserts SBUF/DRAM only.
  - matmul writing bf16 PSUM: asserts "matmul output must be fp32".
  - mm_free=1024: invalid ISA (PSUM bank = 512 fp32 is the matmul max).
  - tree stop-width 1024/2048, fold2 (alias-free), m_group variants: noise.
Engine-time structure (TimelineSim, per core): PE matmuls 227us, ACT PSUM
drains ~485us, DVE col-TT+row-tree ~593us; HW ~740-820us.  The hard wall:
only ACT/DVE can read PSUM (1 elem/cyc effective for ACT, fp32 1x for DVE),
TT min is capped at 2x, and every single-src 4x op (tensor_copy/plain TS)
either cannot reduce or loses its speed with accum_out.
"""

import os

import numpy as np
import ml_dtypes

P = 128
N = 8192
D = 64
KAUG = D + 4  # 68
B = 8

_CACHE = {}

DEFAULT_ROW_MODE = "foldq"


def _build_nc(n=N, mm_free=512, m_group=2048, row_mode="ttr", kaug=KAUG,
              skip_tail=False, repeat=1, col_tail="device"):
    import concourse.bass as bass
    import concourse.mybir as mybir
    import concourse.tile as tile
    from concourse import bacc
    from concourse.masks import make_identity

    fp32 = mybir.dt.float32
    bf16 = mybir.dt.bfloat16
    MIN = mybir.AluOpType.min

    nt_count = n // P          # n-tiles (output partition blocks)
    ngroups = n // m_group     # m groups per n-tile
    mm_per_g = m_group // mm_free

    # Bacc (not raw Bass): its compile pipeline lowers instructions with more
    # sync waits than the ISA's embedded slots into EventSemaphore insts.
    nc = bacc.Bacc("TRN2", target_bir_lowering=False, debug=False)
    xT = nc.dram_tensor("xT", [kaug, n], bf16, kind="ExternalInput")
    yT = nc.dram_tensor("yT", [kaug, n], bf16, kind="ExternalInput")
    out_slots = 3 if row_mode == "tsacc2" else 2
    out = nc.dram_tensor("out", [P, out_slots * nt_count], fp32,
                         kind="ExternalOutput")
    colout = None
    if col_tail == "host":
        # ship the lane-folded col accumulator; host does the 128-lane min
        colout = nc.dram_tensor("colout", [P, n], bf16, kind="ExternalOutput")

    with tile.TileContext(nc) as tc:
        with (
            tc.tile_pool(name="const", bufs=1) as cpool,
            tc.tile_pool(name="work", bufs=3) as wpool,
            tc.tile_pool(name="psum", bufs=2, space="PSUM") as ppool,
        ):
            xTs = cpool.tile([P, n], bf16)
            yTs = cpool.tile([P, n], bf16)
            colacc = cpool.tile([P, n], bf16)
            rowacc = cpool.tile([P, m_group], bf16)
            rowmin = cpool.tile([P, nt_count], fp32)
            if col_tail != "host":
                colmin = cpool.tile([P, nt_count], fp32)
                ident = cpool.tile([P, P], bf16)

            # chunked loads so early matmuls start before the full tensors land
            n_chunks = max(1, n // 2048)
            cw = n // n_chunks
            for c in range(n_chunks):
                nc.sync.dma_start(
                    xTs[:kaug, c * cw:(c + 1) * cw], xT[:, c * cw:(c + 1) * cw])
                nc.sync.dma_start(
                    yTs[:kaug, c * cw:(c + 1) * cw], yT[:, c * cw:(c + 1) * cw])
            if col_tail != "host":
                make_identity(nc, ident)

            if row_mode == "tt":
                rowacc_narrow = cpool.tile([P, mm_free], bf16)
            if row_mode == "ttr2":
                rowacc2 = cpool.tile([P, m_group], bf16)

            if row_mode == "fold2":
                # alias-free variant of "fold": ping-pong col accumulators and
                # alternate row-tree scratch tiles, in case in-place operands
                # demote the DVE from 2x_1P to 1x mode.
                colacc2 = cpool.tile([P, n], bf16)
                accs = [colacc, colacc2]
                vtile = cpool.tile([P, n // 4], bf16)
                for rep in range(repeat):
                    for nt in range(nt_count):
                        lhsT = xTs[:kaug, nt * P:(nt + 1) * P]
                        sfull = wpool.tile([P, n], bf16, tag="s",
                                           name="sfull", bufs=3)
                        for g in range(ngroups):
                            ps = ppool.tile([P, m_group], fp32,
                                            tag="ps", name="ps")
                            for k in range(mm_per_g):
                                nc.tensor.matmul(
                                    ps[:, k * mm_free:(k + 1) * mm_free],
                                    lhsT,
                                    yTs[:kaug,
                                        g * m_group + k * mm_free:
                                        g * m_group + (k + 1) * mm_free],
                                    start=True, stop=True)
                            nc.scalar.copy(
                                out=sfull[:, g * m_group:(g + 1) * m_group],
                                in_=ps)
                        i = (rep * nt_count + nt) % 2
                        if nt == 0 and rep == 0:
                            nc.vector.tensor_copy(out=accs[i], in_=sfull)
                        else:
                            nc.vector.tensor_tensor(
                                out=accs[i], in0=accs[1 - i], in1=sfull,
                                op=MIN)
                        # row fold tree, alternating scratch tiles (no alias)
                        u = wpool.tile([P, n // 2], bf16, tag="u",
                                       name="u", bufs=3)
                        nc.vector.tensor_tensor(
                            out=u, in0=sfull[:, :n // 2],
                            in1=sfull[:, n // 2:], op=MIN)
                        nc.vector.tensor_tensor(
                            out=vtile, in0=u[:, :n // 4],
                            in1=u[:, n // 4:], op=MIN)
                        nc.vector.tensor_tensor(
                            out=u[:, :n // 8], in0=vtile[:, :n // 8],
                            in1=vtile[:, n // 8:], op=MIN)
                        nc.vector.tensor_tensor(
                            out=vtile[:, :n // 16], in0=u[:, :n // 16],
                            in1=u[:, n // 16:n // 8], op=MIN)
                        nc.vector.tensor_reduce(
                            out=rowmin[:, nt:nt + 1], in_=vtile[:, :n // 16],
                            axis=mybir.AxisListType.X, op=MIN)
                final_colacc = accs[(repeat * nt_count - 1) % 2]
            else:
                final_colacc = colacc

            if row_mode == "tsacc2":
                # tsacc + ACT/DVE drain split: every drain_period-th n-tile,
                # the last m-group is drained from PSUM by a DVE
                # tensor_scalar (1x from PSUM, converts to bf16 into sfull
                # AND min-accumulates that group's row partial into rowmin2)
                # instead of the busier ACT engine; host takes
                # min(rowmin, rowmin2).
                drain_period = int(os.environ.get("CHAMFER_DRAIN_PERIOD",
                                                  "4"))
                drain_g = int(os.environ.get("CHAMFER_DRAIN_G", "0"))
                rowmin2 = cpool.tile([P, nt_count], fp32)
                nc.vector.memset(rowmin2, 3.0e38)
                for rep in range(repeat):
                    for nt in range(nt_count):
                        drain = (nt % drain_period == drain_period - 1)
                        lhsT = xTs[:kaug, nt * P:(nt + 1) * P]
                        sfull = wpool.tile([P, n], bf16, tag="s",
                                           name="sfull", bufs=3)
                        for g in range(ngroups):
                            ps = ppool.tile([P, m_group], fp32,
                                            tag="ps", name="ps")
                            for k in range(mm_per_g):
                                nc.tensor.matmul(
                                    ps[:, k * mm_free:(k + 1) * mm_free],
                                    lhsT,
                                    yTs[:kaug,
                                        g * m_group + k * mm_free:
                                        g * m_group + (k + 1) * mm_free],
                                    start=True, stop=True)
                            if drain and g == drain_g:
                                nc.vector.tensor_scalar(
                                    out=sfull[:, g * m_group:
                                              (g + 1) * m_group],
                                    in0=ps, scalar1=3.0e38, scalar2=None,
                                    op0=MIN, op1=MIN,
                                    accum_out=rowmin2[:, nt:nt + 1])
                            else:
                                nc.scalar.copy(
                                    out=sfull[:, g * m_group:
                                              (g + 1) * m_group],
                                    in_=ps)
                        # main row-TS covers the ACT-copied groups, which
                        # are contiguous only for drain_g == 0 or last
                        ts_lo = (m_group if (drain and drain_g == 0) else 0)
                        ts_hi = n - (m_group
                                     if (drain and drain_g == ngroups - 1)
                                     else 0)
                        if nt == 0 and rep == 0:
                            nc.vector.tensor_scalar(
                                out=colacc, in0=sfull, scalar1=3.0e38,
                                scalar2=None, op0=MIN, op1=MIN,
                                accum_out=rowmin[:, nt:nt + 1])
                        else:
                            nc.vector.tensor_tensor(
                                out=colacc, in0=colacc, in1=sfull, op=MIN)
                            scr = wpool.tile([P, n], bf16, tag="scr",
                                             name="scr", bufs=2)
                            nc.vector.tensor_scalar(
                                out=scr[:, ts_lo:ts_hi],
                                in0=sfull[:, ts_lo:ts_hi],
                                scalar1=3.0e38, scalar2=None,
                                op0=MIN, op1=MIN,
                                accum_out=rowmin[:, nt:nt + 1])

            if row_mode == "foldx":
                # "fold" with HW-measured rebalancing knobs:
                #   CHAMFER_DVEDRAIN_PERIOD=k: every k-th n-tile, group 0 is
                #     drained PSUM->SBUF by DVE tensor_copy (HW: ~2us/group,
                #     faster than the erratum-slowed ACT ~3us) instead of ACT.
                #   CHAMFER_POOL_COLG=1: gpsimd does the col-min TT for the
                #     last m-group (frees ~1.1us/tile of DVE).
                #   CHAMFER_POOL_TREE=1: gpsimd does the 1024- and 512-wide
                #     row-tree levels (frees ~0.9us/tile of DVE).
                dve_period = int(os.environ.get(
                    "CHAMFER_DVEDRAIN_PERIOD", "0"))
                pool_colg = int(os.environ.get("CHAMFER_POOL_COLG", "0"))
                pool_tree = int(os.environ.get("CHAMFER_POOL_TREE", "0"))
                for rep in range(repeat):
                    for nt in range(nt_count):
                        dve_drain = dve_period and (
                            nt % dve_period == dve_period - 1)
                        lhsT = xTs[:kaug, nt * P:(nt + 1) * P]
                        sfull = wpool.tile([P, n], bf16, tag="s",
                                           name="sfull", bufs=3)
                        for g in range(ngroups):
                            ps = ppool.tile([P, m_group], fp32,
                                            tag="ps", name="ps")
                            for k in range(mm_per_g):
                                nc.tensor.matmul(
                                    ps[:, k * mm_free:(k + 1) * mm_free],
                                    lhsT,
                                    yTs[:kaug,
                                        g * m_group + k * mm_free:
                                        g * m_group + (k + 1) * mm_free],
                                    start=True, stop=True)
                            gsl = sfull[:, g * m_group:(g + 1) * m_group]
                            if dve_drain and g == 0:
                                nc.vector.tensor_copy(out=gsl, in_=ps)
                            else:
                                nc.scalar.copy(out=gsl, in_=ps)
                        # col-min accumulate
                        if nt == 0 and rep == 0:
                            nc.vector.tensor_copy(out=colacc, in_=sfull)
                        elif pool_colg:
                            csp = n - m_group
                            nc.vector.tensor_tensor(
                                out=colacc[:, :csp], in0=colacc[:, :csp],
                                in1=sfull[:, :csp], op=MIN)
                            nc.gpsimd.tensor_tensor(
                                out=colacc[:, csp:], in0=colacc[:, csp:],
                                in1=sfull[:, csp:], op=MIN)
                        else:
                            nc.vector.tensor_tensor(
                                out=colacc, in0=colacc, in1=sfull, op=MIN)
                        # row fold tree
                        u = wpool.tile([P, n // 2], bf16, tag="u",
                                       name="u", bufs=3)
                        nc.vector.tensor_tensor(
                            out=u, in0=sfull[:, :n // 2],
                            in1=sfull[:, n // 2:], op=MIN)
                        nc.vector.tensor_tensor(
                            out=u[:, :n // 4], in0=u[:, :n // 4],
                            in1=u[:, n // 4:n // 2], op=MIN)
                        w = n // 4
                        while w > 512:
                            eng = nc.gpsimd if pool_tree else nc.vector
                            eng.tensor_tensor(
                                out=u[:, :w // 2], in0=u[:, :w // 2],
                                in1=u[:, w // 2:w], op=MIN)
                            w //= 2
                        nc.vector.tensor_reduce(
                            out=rowmin[:, nt:nt + 1], in_=u[:, :w],
                            axis=mybir.AxisListType.X, op=MIN)

            if row_mode == "tsacc":
                # One n-wide s tile per n-tile: ONE wide col-min TT (2x) and
                # row mins via ONE tensor_scalar with min-accumulate (4x:
                # TensorScalarPtr supports 4x_2p; out=min(s,3e38)=s is a
                # throwaway wide write, accum_out = min over the free dim).
                for rep in range(repeat):
                    for nt in range(nt_count):
                        lhsT = xTs[:kaug, nt * P:(nt + 1) * P]
                        sfull = wpool.tile([P, n], bf16, tag="s",
                                           name="sfull", bufs=3)
                        for g in range(ngroups):
                            ps = ppool.tile([P, m_group], fp32,
                                            tag="ps", name="ps")
                            for k in range(mm_per_g):
                                nc.tensor.matmul(
                                    ps[:, k * mm_free:(k + 1) * mm_free],
                                    lhsT,
                                    yTs[:kaug,
                                        g * m_group + k * mm_free:
                                        g * m_group + (k + 1) * mm_free],
                                    start=True, stop=True)
                            nc.scalar.copy(
                                out=sfull[:, g * m_group:(g + 1) * m_group],
                                in_=ps)
                        if nt == 0 and rep == 0:
                            # TS both initializes colacc (out = min(s, 3e38)
                            # = s) and produces this tile's row min.
                            nc.vector.tensor_scalar(
                                out=colacc, in0=sfull, scalar1=3.0e38,
                                scalar2=None, op0=MIN, op1=MIN,
                                accum_out=rowmin[:, nt:nt + 1])
                        else:
                            nc.vector.tensor_tensor(
                                out=colacc, in0=colacc, in1=sfull, op=MIN)
                            scr = wpool.tile([P, n], bf16, tag="scr",
                                             name="scr", bufs=2)
                            nc.vector.tensor_scalar(
                                out=scr, in0=sfull, scalar1=3.0e38,
                                scalar2=None, op0=MIN, op1=MIN,
                                accum_out=rowmin[:, nt:nt + 1])

            if row_mode == "foldq":
                # "fold" with the row fold-trees of Q consecutive n-tiles
                # JOINED into one strided-AP instruction per tree level
                # (same element count at 2x, but 1/Q-th the DVE instruction
                # overhead, which HW shows is ~0.4-0.9us per op) and ONE
                # reduce producing Q rowmins at once.
                Q = int(os.environ.get("CHAMFER_Q", "2"))
                defer = int(os.environ.get("CHAMFER_DEFER", "4"))
                dve_period = int(os.environ.get(
                    "CHAMFER_DVEDRAIN_PERIOD", "0"))
                nq = nt_count // Q
                if defer:
                    # pending remnants: DEFER quads x Q tiles x 2048 each
                    pend = cpool.tile([P, defer * Q * 2048], bf16)
                    pv = pend.rearrange("p (d w) -> p d w", d=defer * Q)
                for rep in range(repeat):
                    for qi in range(nq):
                        squad = wpool.tile([P, Q * n], bf16, tag="sq",
                                           name="squad", bufs=2)
                        for t in range(Q):
                            nt = qi * Q + t
                            lhsT = xTs[:kaug, nt * P:(nt + 1) * P]
                            for g in range(ngroups):
                                ps = ppool.tile([P, m_group], fp32,
                                                tag="ps", name="ps")
                                for k in range(mm_per_g):
                                    nc.tensor.matmul(
                                        ps[:, k * mm_free:(k + 1) * mm_free],
                                        lhsT,
                                        yTs[:kaug,
                                            g * m_group + k * mm_free:
                                            g * m_group + (k + 1) * mm_free],
                                        start=True, stop=True)
                                gout = squad[:, t * n + g * m_group:
                                             t * n + (g + 1) * m_group]
                                if (dve_period and g == 0
                                        and nt % dve_period
                                        == dve_period - 1):
                                    nc.vector.tensor_copy(out=gout, in_=ps)
                                else:
                                    nc.scalar.copy(out=gout, in_=ps)
                            # col-min accumulate per sub-tile (fine-grained
                            # so it overlaps the next sub-tile's drains)
                            ssl = squad[:, t * n:(t + 1) * n]
                            if qi == 0 and t == 0 and rep == 0:
                                nc.vector.tensor_copy(out=colacc, in_=ssl)
                            else:
                                nc.vector.tensor_tensor(
                                    out=colacc, in0=colacc, in1=ssl, op=MIN)
                        # joint row fold tree over all Q sub-tiles
                        u = wpool.tile([P, Q * (n // 2)], bf16, tag="u",
                                       name="u", bufs=2)
                        sv = squad.rearrange("p (q w) -> p q w", q=Q)
                        uv = u.rearrange("p (q w) -> p q w", q=Q)
                        nc.vector.tensor_tensor(
                            out=uv[:, :, 0:n // 2], in0=sv[:, :, 0:n // 2],
                            in1=sv[:, :, n // 2:n], op=MIN)
                        if defer:
                            # L2 writes this quad's [Q, 2048] remnant into
                            # the pending buffer; every `defer` quads, the
                            # narrow levels + reduce run ONCE over all of
                            # them (fewer, wider DVE ops).
                            slot = (qi % defer) * Q
                            nc.vector.tensor_tensor(
                                out=pv[:, slot:slot + Q, :],
                                in0=uv[:, :, 0:2048],
                                in1=uv[:, :, 2048:4096], op=MIN)
                            if qi % defer == defer - 1:
                                w = 2048
                                while w > 512:
                                    nc.vector.tensor_tensor(
                                        out=pv[:, :, 0:w // 2],
                                        in0=pv[:, :, 0:w // 2],
                                        in1=pv[:, :, w // 2:w], op=MIN)
                                    w //= 2
                                lo = (qi - defer + 1) * Q
                                nc.vector.tensor_reduce(
                                    out=rowmin[:, lo:lo + defer * Q],
                                    in_=pv[:, :, 0:w],
                                    axis=mybir.AxisListType.X, op=MIN)
                        else:
                            w = n // 2
                            while w > 512:
                                nc.vector.tensor_tensor(
                                    out=uv[:, :, 0:w // 2],
                                    in0=uv[:, :, 0:w // 2],
                                    in1=uv[:, :, w // 2:w], op=MIN)
                                w //= 2
                            nc.vector.tensor_reduce(
                                out=rowmin[:, qi * Q:(qi + 1) * Q],
                                in_=uv[:, :, 0:w],
                                axis=mybir.AxisListType.X, op=MIN)

            if row_mode == "fold":
                # One n-wide s tile per n-tile: ONE wide col-min TT, and row
                # mins via a fold tree of wide TT-mins + one small reduce.
                for rep in range(repeat):
                    for nt in range(nt_count):
                        lhsT = xTs[:kaug, nt * P:(nt + 1) * P]
                        sfull = wpool.tile([P, n], bf16, tag="s",
                                           name="sfull", bufs=3)
                        for g in range(ngroups):
                            ps = ppool.tile([P, m_group], fp32,
                                            tag="ps", name="ps")
                            for k in range(mm_per_g):
                                nc.tensor.matmul(
                                    ps[:, k * mm_free:(k + 1) * mm_free],
                                    lhsT,
                                    yTs[:kaug,
                                        g * m_group + k * mm_free:
                                        g * m_group + (k + 1) * mm_free],
                                    start=True, stop=True)
                            nc.scalar.copy(
                                out=sfull[:, g * m_group:(g + 1) * m_group],
                                in_=ps)
                        if nt == 0 and rep == 0:
                            nc.vector.tensor_copy(out=colacc, in_=sfull)
                        else:
                            nc.vector.tensor_tensor(
                                out=colacc, in0=colacc, in1=sfull, op=MIN)
                        # row fold tree
                        u = wpool.tile([P, n // 2], bf16, tag="u",
                                       name="u", bufs=3)
                        nc.vector.tensor_tensor(
                            out=u, in0=sfull[:, :n // 2],
                            in1=sfull[:, n // 2:], op=MIN)
                        w = n // 2
                        stop_w = int(os.environ.get("CHAMFER_TREE_STOP",
                                                    "512"))
                        while w > stop_w:
                            nc.vector.tensor_tensor(
                                out=u[:, :w // 2], in0=u[:, :w // 2],
                                in1=u[:, w // 2:w], op=MIN)
                            w //= 2
                        nc.vector.tensor_reduce(
                            out=rowmin[:, nt:nt + 1], in_=u[:, :w],
                            axis=mybir.AxisListType.X, op=MIN)

            for rep in range(
                    repeat if row_mode not in ("fold", "fold2", "tsacc",
                                               "tsacc2", "foldx", "foldq")
                    else 0):
              for nt in range(nt_count):
                lhsT = xTs[:kaug, nt * P:(nt + 1) * P]
                for g in range(ngroups):
                    ps = ppool.tile([P, m_group], fp32, tag="ps", name="ps")
                    for k in range(mm_per_g):
                        nc.tensor.matmul(
                            ps[:, k * mm_free:(k + 1) * mm_free],
                            lhsT,
                            yTs[:kaug, g * m_group + k * mm_free:
                                g * m_group + (k + 1) * mm_free],
                            start=True,
                            stop=True,
                        )
                    s = wpool.tile([P, m_group], bf16, name="s")
                    nc.scalar.copy(out=s, in_=ps)

                    # column-min accumulator (n folded into the 128 lanes)
                    csl = colacc[:, g * m_group:(g + 1) * m_group]
                    if nt == 0:
                        nc.vector.tensor_copy(out=csl, in_=s)
                    else:
                        nc.vector.tensor_tensor(out=csl, in0=csl, in1=s, op=MIN)

                    # row mins
                    if row_mode == "ttr2":
                        # like "ttr" but ping-pongs the elementwise-min
                        # accumulator to avoid in-place out/in1 aliasing
                        accs = [rowacc, rowacc2]
                        dst = accs[g % 2]
                        src = s if g == 0 else accs[1 - g % 2]
                        nc.vector.tensor_tensor_reduce(
                            out=dst,
                            in0=s,
                            in1=src,
                            scale=1.0,
                            scalar=3.0e38,
                            op0=MIN,
                            op1=MIN,
                            accum_out=rowmin[:, nt:nt + 1],
                        )
                    elif row_mode == "ttr":
                        # rowacc = min(rowacc, s) elementwise; accum_out gets
                        # min over the free dim of the updated rowacc. The
                        # last group's accum covers all m -> true row min.
                        nc.vector.tensor_tensor_reduce(
                            out=rowacc,
                            in0=s,
                            in1=(s if g == 0 else rowacc),
                            scale=1.0,
                            scalar=3.0e38,
                            op0=MIN,
                            op1=MIN,
                            accum_out=rowmin[:, nt:nt + 1],
                        )
                    else:
                        for k in range(mm_per_g):
                            ssl = s[:, k * mm_free:(k + 1) * mm_free]
                            if g == 0 and k == 0:
                                nc.vector.tensor_copy(out=rowacc_narrow, in_=ssl)
                            else:
                                nc.vector.tensor_tensor(
                                    out=rowacc_narrow, in0=rowacc_narrow,
                                    in1=ssl, op=MIN)
                        if g == ngroups - 1:
                            nc.vector.tensor_reduce(
                                out=rowmin[:, nt:nt + 1], in_=rowacc_narrow,
                                axis=mybir.AxisListType.X, op=MIN)

            # column-min finish: transpose each [128, 128] block of colacc on
            # PE, then min-reduce the (former partition) lanes on DVE.
            if col_tail == "host":
                nc.sync.dma_start(colout[:, :], final_colacc[:, :])
            elif not skip_tail:
                # batch transposes into wide bf16 PSUM tiles so the lane-min
                # runs as a few wide DVE reduces instead of nt_count small ones
                tpb = max(1, min(nt_count, (m_group * 2) // P))
                for t0 in range(0, nt_count, tpb):
                    cnt = min(tpb, nt_count - t0)
                    pt = ppool.tile([P, tpb, P], bf16, tag="ps", name="pt")
                    for i in range(cnt):
                        t = t0 + i
                        nc.tensor.transpose(
                            pt[:, i, :], final_colacc[:, t * P:(t + 1) * P], ident)
                    nc.vector.tensor_reduce(
                        out=colmin[:, t0:t0 + cnt], in_=pt[:, :cnt, :],
                        axis=mybir.AxisListType.X, op=MIN)
            else:
                nc.vector.tensor_copy(out=colmin, in_=rowmin)

            nc.sync.dma_start(out[:, :nt_count], rowmin[:, :])
            if row_mode == "tsacc2":
                nc.sync.dma_start(
                    out[:, nt_count:2 * nt_count], rowmin2[:, :])
            if col_tail != "host":
                nc.sync.dma_start(
                    out[:, (out_slots - 1) * nt_count:], colmin[:, :])

    nc.finalize()  # runs the Bacc compile passes (event sems, reg alloc, ...)
    return nc


def _prep_inputs(x, y, kaug=KAUG):
    """Build the augmented, transposed bf16 operands for each batch."""
    bf = ml_dtypes.bfloat16
    in_maps = []
    for b in range(x.shape[0]):
        xb = np.asarray(x[b], dtype=np.float32)
        yb = np.asarray(y[b], dtype=np.float32)
        n = xb.shape[0]
        x2 = np.sum(xb * xb, axis=-1)
        y2 = np.sum(yb * yb, axis=-1)
        x2_hi = x2.astype(bf)
        x2_lo = (x2 - x2_hi.astype(np.float32)).astype(bf)
        y2_hi = y2.astype(bf)
        y2_lo = (y2 - y2_hi.astype(np.float32)).astype(bf)
        ones = np.ones((1, n), dtype=bf)
        xT = np.concatenate(
            [xb.T.astype(bf), ones, ones, x2_hi[None], x2_lo[None]], axis=0)
        yT = np.concatenate(
            [(-2.0 * yb).T.astype(bf), y2_hi[None], y2_lo[None], ones, ones],
            axis=0)
        if kaug > KAUG:
            pad = np.zeros((kaug - KAUG, n), dtype=bf)
            xT = np.concatenate([xT, pad], axis=0)
            yT = np.concatenate([yT, pad], axis=0)
        in_maps.append({
            "xT": np.ascontiguousarray(xT),
            "yT": np.ascontiguousarray(yT),
        })
    return in_maps


def _postprocess(results, n=N):
    nt_count = n // P
    total = 0.0
    nb = len(results)
    for b in range(nb):
        o = np.asarray(results[b]["out"], dtype=np.float64)
        rowmin = o[:, :nt_count].T.reshape(-1)   # [n], index t*128+p
        if o.shape[1] == 3 * nt_count:
            # tsacc2: second slot holds DVE-drained group row partials
            rowmin = np.minimum(
                rowmin, o[:, nt_count:2 * nt_count].T.reshape(-1))
        if "colout" in results[b]:
            co = np.asarray(results[b]["colout"], dtype=np.float32)
            colmin = co.min(axis=0).astype(np.float64)
        else:
            colmin = o[:, -nt_count:].T.reshape(-1)
        total += np.sqrt(np.maximum(rowmin, 0.0)).sum()
        total += np.sqrt(np.maximum(colmin, 0.0)).sum()
    loss = total / nb / n
    return np.asarray(loss, dtype=np.float32)


def _get_runner(n_cores=B):
    """Build the Bass module once and return a reusable jitted runner.

    Modeled on concourse.bass2jax.run_bass_via_pjrt's multi-core branch, but
    keeps the jitted callable so repeated invocations don't re-lower."""
    key = ("runner", n_cores)
    if key in _CACHE:
        return _CACHE[key]

    import jax
    from jax.experimental.shard_map import shard_map
    from jax.sharding import Mesh, PartitionSpec
    from concourse import bass2jax, mybir

    nc = _build_nc(row_mode=os.environ.get("CHAMFER_ROW_MODE",
                                           DEFAULT_ROW_MODE),
                   col_tail=os.environ.get("CHAMFER_COL_TAIL", "device"))

    bass2jax.install_neuronx_cc_hook()
    assert nc.dbg_addr is None

    partition_name = (
        nc.partition_id_tensor.name if nc.partition_id_tensor else None)
    in_names, out_names, out_avals = [], [], []
    for alloc in nc.m.functions[0].allocations:
        if not isinstance(alloc, mybir.MemoryLocationSet):
            continue
        name = alloc.memorylocations[0].name
        if alloc.kind == "ExternalInput":
            if name != partition_name:
                in_names.append(name)
        elif alloc.kind == "ExternalOutput":
            out_names.append(name)
            out_avals.append(jax.core.ShapedArray(
                tuple(alloc.tensor_shape), mybir.dt.np(alloc.dtype)))
    n_params = len(in_names)
    n_outs = len(out_avals)
    all_in_names = list(in_names) + list(out_names)
    if partition_name is not None:
        all_in_names.append(partition_name)
    donate = tuple(range(n_params, n_params + n_outs))

    def _body(*args):
        operands = list(args)
        if partition_name is not None:
            operands.append(bass2jax.partition_id_tensor())
        outs = bass2jax._bass_exec_p.bind(
            *operands,
            out_avals=tuple(out_avals),
            in_names=tuple(all_in_names),
            out_names=tuple(out_names),
            lowering_input_output_aliases=(),
            sim_require_finite=True,
            sim_require_nnan=True,
            nc=nc,
        )
        return tuple(outs)

    devices = jax.devices()[:n_cores]
    mesh = Mesh(np.asarray(devices), ("core",))
    sharded = jax.jit(
        shard_map(
            _body, mesh=mesh,
            in_specs=(PartitionSpec("core"),) * (n_params + n_outs),
            out_specs=(PartitionSpec("core"),) * n_outs,
            check_rep=False,
        ),
        donate_argnums=donate,
        keep_unused=True,
    )

    def run(in_maps):
        per_core = [[np.asarray(m[nm]) for nm in in_names] for m in in_maps]
        concat_in = [
            np.concatenate([per_core[c][i] for c in range(n_cores)], axis=0)
            for i in range(n_params)
        ]
        concat_zeros = [
            np.zeros((n_cores * a.shape[0], *a.shape[1:]), a.dtype)
            for a in out_avals
        ]
        out_arrs = sharded(*concat_in, *concat_zeros)
        jax.block_until_ready(out_arrs)
        return [
            {nm: np.asarray(out_arrs[i]).reshape(
                n_cores, *out_avals[i].shape)[c]
             for i, nm in enumerate(out_names)}
            for c in range(n_cores)
        ]

    _CACHE[key] = run
    return run


def kernel(x, y):
    import time

    x = np.asarray(x)
    y = np.asarray(y)
    in_maps = _prep_inputs(x, y)
    run = _get_runner(n_cores=len(in_maps))
    # the device occasionally wedges transiently on a fresh NEFF's first
    # execution (NRT_EXEC_UNIT_UNRECOVERABLE); a retry reliably clears it
    last_err = None
    for attempt in range(4):
        try:
            results = run(in_maps)
            return _postprocess(results)
        except Exception as e:  # noqa: BLE001 - retry any runtime failure
            last_err = e
            time.sleep(2.0)
            try:
                import jax
                jax.clear_caches()
            except Exception:
                pass
            _CACHE.clear()  # rebuild runner; NEFF recompile is disk-cached
            run = _get_runner(n_cores=len(in_maps))
    raise last_err



# revision 30
# speedup vs baseline: 1.6922x; 1.0552x over previous
"""Chamfer loss kernel for Trainium2 (8 NeuronCores, one batch per core).

Problem: B=8, N=M=8192, D=64 fp32.
  rd = pairwise euclidean distances x[b] vs y[b]   [B, N, M]
  loss = mean_b( sum_n min_m rd + sum_m min_n rd ) / M

Device strategy (per core = one batch):
  - sqrt is monotonic -> only need minima of SQUARED distances; sqrt+sums
    happen on host over 2*8192 values per batch.
  - d2 = x2 + y2 - 2*x.y is produced entirely by ONE bf16 matmul with an
    augmented contraction dim:
       lhsT rows (x side, [68, N]): [x_d (64) ; 1 ; 1 ; x2_hi ; x2_lo]
       rhs  rows (y side, [68, M]): [-2*y_d (64) ; y2_hi ; y2_lo ; 1 ; 1]
    so psum = sum_d x_d*(-2 y_d) + y2_hi + y2_lo + x2_hi + x2_lo = d2.
    (hi/lo bf16 splits keep the squared-norm terms at ~fp24 precision.)
  - ScalarE copies each PSUM group to one n-wide bf16 SBUF tile; VectorE
    (the bottleneck engine, bf16 tensor_tensor min at 2 elem/cycle/lane)
    then does per n-tile: ONE wide col-min accumulate into a [128, M]
    accumulator (n folded mod 128) + a fold-tree of wide TT-mins and one
    small reduce for the row mins.
  - The col accumulator is finished by PE transposes + wide DVE reduces.
Host does the final sqrt / sums / mean in float64.
(tensor_tensor_reduce / tensor_tensor_scan were evaluated: TTR faults this
runtime (NRT_EXEC_UNIT_UNRECOVERABLE), scan is ~2.5x slower than the tree.)

Default mode "foldq" = "fold" with two instruction-count reductions that
measure at-or-slightly-better than fold on HW (never worse, bit-identical
result): (1) the row fold-trees of Q=2 consecutive n-tiles are JOINED into
one strided-AP TT per level ([P, Q, w] views, innermost step 1 keeps the
2x DVE mode), and (2) tree levels below 4096 are deferred into a pending
buffer and finished ONCE per 4 quads (8 tiles) with one TT per level and a
single [P, 8, 512] reduce producing 8 rowmins. DVE instruction count per
8 tiles: 40 -> 19.

Alternatives evaluated on HW (all lose to "fold"; measured with interleaved
129-long chained-submission benches, bench.py):
  - tsacc: row min via ONE tensor_scalar(op0=min, op1=min, accum_out):
    the cost model claims 4x_2p for TensorScalarPtr+accum, but ON HW the
    accumulate path runs at ~1x (7.8-10.2us per [128,8192] op vs 2.7-3.0us
    without accum) -> ~25% slower than fold overall.  The plain-TS 4x is
    real; the accumulator kills it.
  - tsacc2: tsacc + rotating DVE-drain of one PSUM group: worse still
    (scheduling bubbles).
  - foldx@dd: draining some PSUM groups via DVE tensor_copy (micro-bench
    says ~2us/group vs ACT ~3us) is a wash at dd=4 and worse at dd=2.
  - gpsimd (Pool) TensorTensor: rejected by walrus codegen
    ("Instruction engine check failed (Pool)") - no Pool offload possible.
  - gpsimd-initiated DMA with accum_op=min (SBUF->SBUF): rejected by the
    BIR verifier (visitInstDMACopy throws).
  - DMA cast fp32->bf16 from PSUM: dma_start asserts SBUF/DRAM only.
  - matmul writing bf16 PSUM: asserts "matmul output must be fp32".
  - mm_free=1024: invalid ISA (PSUM bank = 512 fp32 is the matmul max).
  - tree stop-width 1024/2048, fold2 (alias-free), m_group variants: noise.
Engine-time structure (TimelineSim, per core): PE matmuls 227us, ACT PSUM
drains ~485us, DVE col-TT+row-tree ~593us; HW ~740-820us.  The hard wall:
only ACT/DVE can read PSUM (1 elem/cyc effective for ACT, fp32 1x for DVE),
TT min is capped at 2x, and every single-src 4x op (tensor_copy/plain TS)
either cannot reduce or loses its speed with accum_out.
"""

import os

import numpy as np
import ml_dtypes

P = 128
N = 8192
D = 64
KAUG = D + 4  # 68
B = 8

_CACHE = {}

DEFAULT_ROW_MODE = "foldq"


def _build_nc(n=N, mm_free=512, m_group=2048, row_mode="ttr", kaug=KAUG,
              skip_tail=False, repeat=1, col_tail="device"):
    import concourse.bass as bass
    import concourse.mybir as mybir
    import concourse.tile as tile
    from concourse import bacc
    from concourse.masks import make_identity

    fp32 = mybir.dt.float32
    bf16 = mybir.dt.bfloat16
    MIN = mybir.AluOpType.min

    nt_count = n // P          # n-tiles (output partition blocks)
    ngroups = n // m_group     # m groups per n-tile
    mm_per_g = m_group // mm_free

    # Bacc (not raw Bass): its compile pipeline lowers instructions with more
    # sync waits than the ISA's embedded slots into EventSemaphore insts.
    nc = bacc.Bacc("TRN2", target_bir_lowering=False, debug=False)
    xT = nc.dram_tensor("xT", [kaug, n], bf16, kind="ExternalInput")
    yT = nc.dram_tensor("yT", [kaug, n], bf16, kind="ExternalInput")
    out_slots = 3 if row_mode == "tsacc2" else 2
    out = nc.dram_tensor("out", [P, out_slots * nt_count], fp32,
                         kind="ExternalOutput")
    colout = None
    if col_tail == "host":
        # ship the lane-folded col accumulator; host does the 128-lane min
        colout = nc.dram_tensor("colout", [P, n], bf16, kind="ExternalOutput")

    with tile.TileContext(nc) as tc:
        with (
            tc.tile_pool(name="const", bufs=1) as cpool,
            tc.tile_pool(name="work", bufs=3) as wpool,
            tc.tile_pool(name="psum", bufs=2, space="PSUM") as ppool,
        ):
            xTs = cpool.tile([P, n], bf16)
            yTs = cpool.tile([P, n], bf16)
            colacc = cpool.tile([P, n], bf16)
            rowacc = cpool.tile([P, m_group], bf16)
            rowmin = cpool.tile([P, nt_count], fp32)
            if col_tail != "host":
                colmin = cpool.tile([P, nt_count], fp32)
                ident = cpool.tile([P, P], bf16)

            # chunked loads so early matmuls start before the full tensors land
            n_chunks = max(1, n // 2048)
            cw = n // n_chunks
            for c in range(n_chunks):
                nc.sync.dma_start(
                    xTs[:kaug, c * cw:(c + 1) * cw], xT[:, c * cw:(c + 1) * cw])
                nc.sync.dma_start(
                    yTs[:kaug, c * cw:(c + 1) * cw], yT[:, c * cw:(c + 1) * cw])
            if col_tail != "host":
                make_identity(nc, ident)

            if row_mode == "tt":
                rowacc_narrow = cpool.tile([P, mm_free], bf16)
            if row_mode == "ttr2":
                rowacc2 = cpool.tile([P, m_group], bf16)

            if row_mode == "fold2":
                # alias-free variant of "fold": ping-pong col accumulators and
                # alternate row-tree scratch tiles, in case in-place operands
                # demote the DVE from 2x_1P to 1x mode.
                colacc2 = cpool.tile([P, n], bf16)
                accs = [colacc, colacc2]
                vtile = cpool.tile([P, n // 4], bf16)
                for rep in range(repeat):
                    for nt in range(nt_count):
                        lhsT = xTs[:kaug, nt * P:(nt + 1) * P]
                        sfull = wpool.tile([P, n], bf16, tag="s",
                                           name="sfull", bufs=3)
                        for g in range(ngroups):
                            ps = ppool.tile([P, m_group], fp32,
                                            tag="ps", name="ps")
                            for k in range(mm_per_g):
                                nc.tensor.matmul(
                                    ps[:, k * mm_free:(k + 1) * mm_free],
                                    lhsT,
                                    yTs[:kaug,
                                        g * m_group + k * mm_free:
                                        g * m_group + (k + 1) * mm_free],
                                    start=True, stop=True)
                            nc.scalar.copy(
                                out=sfull[:, g * m_group:(g + 1) * m_group],
                                in_=ps)
                        i = (rep * nt_count + nt) % 2
                        if nt == 0 and rep == 0:
                            nc.vector.tensor_copy(out=accs[i], in_=sfull)
                        else:
                            nc.vector.tensor_tensor(
                                out=accs[i], in0=accs[1 - i], in1=sfull,
                                op=MIN)
                        # row fold tree, alternating scratch tiles (no alias)
                        u = wpool.tile([P, n // 2], bf16, tag="u",
                                       name="u", bufs=3)
                        nc.vector.tensor_tensor(
                            out=u, in0=sfull[:, :n // 2],
                            in1=sfull[:, n // 2:], op=MIN)
                        nc.vector.tensor_tensor(
                            out=vtile, in0=u[:, :n // 4],
                            in1=u[:, n // 4:], op=MIN)
                        nc.vector.tensor_tensor(
                            out=u[:, :n // 8], in0=vtile[:, :n // 8],
                            in1=vtile[:, n // 8:], op=MIN)
                        nc.vector.tensor_tensor(
                            out=vtile[:, :n // 16], in0=u[:, :n // 16],
                            in1=u[:, n // 16:n // 8], op=MIN)
                        nc.vector.tensor_reduce(
                            out=rowmin[:, nt:nt + 1], in_=vtile[:, :n // 16],
                            axis=mybir.AxisListType.X, op=MIN)
                final_colacc = accs[(repeat * nt_count - 1) % 2]
            else:
                final_colacc = colacc

            if row_mode == "tsacc2":
                # tsacc + ACT/DVE drain split: every drain_period-th n-tile,
                # the last m-group is drained from PSUM by a DVE
                # tensor_scalar (1x from PSUM, converts to bf16 into sfull
                # AND min-accumulates that group's row partial into rowmin2)
                # instead of the busier ACT engine; host takes
                # min(rowmin, rowmin2).
                drain_period = int(os.environ.get("CHAMFER_DRAIN_PERIOD",
                                                  "4"))
                drain_g = int(os.environ.get("CHAMFER_DRAIN_G", "0"))
                rowmin2 = cpool.tile([P, nt_count], fp32)
                nc.vector.memset(rowmin2, 3.0e38)
                for rep in range(repeat):
                    for nt in range(nt_count):
                        drain = (nt % drain_period == drain_period - 1)
                        lhsT = xTs[:kaug, nt * P:(nt + 1) * P]
                        sfull = wpool.tile([P, n], bf16, tag="s",
                                           name="sfull", bufs=3)
                        for g in range(ngroups):
                            ps = ppool.tile([P, m_group], fp32,
                                            tag="ps", name="ps")
                            for k in range(mm_per_g):
                                nc.tensor.matmul(
                                    ps[:, k * mm_free:(k + 1) * mm_free],
                                    lhsT,
                                    yTs[:kaug,
                                        g * m_group + k * mm_free:
                                        g * m_group + (k + 1) * mm_free],
                                    start=True, stop=True)
                            if drain and g == drain_g:
                                nc.vector.tensor_scalar(
                                    out=sfull[:, g * m_group:
                                              (g + 1) * m_group],
                                    in0=ps, scalar1=3.0e38, scalar2=None,
                                    op0=MIN, op1=MIN,
                                    accum_out=rowmin2[:, nt:nt + 1])
                            else:
                                nc.scalar.copy(
                                    out=sfull[:, g * m_group:
                                              (g + 1) * m_group],
                                    in_=ps)
                        # main row-TS covers the ACT-copied groups, which
                        # are contiguous only for drain_g == 0 or last
                        ts_lo = (m_group if (drain and drain_g == 0) else 0)
                        ts_hi = n - (m_group
                                     if (drain and drain_g == ngroups - 1)
                                     else 0)
                        if nt == 0 and rep == 0:
                            nc.vector.tensor_scalar(
                                out=colacc, in0=sfull, scalar1=3.0e38,
                                scalar2=None, op0=MIN, op1=MIN,
                                accum_out=rowmin[:, nt:nt + 1])
                        else:
                            nc.vector.tensor_tensor(
                                out=colacc, in0=colacc, in1=sfull, op=MIN)
                            scr = wpool.tile([P, n], bf16, tag="scr",
                                             name="scr", bufs=2)
                            nc.vector.tensor_scalar(
                                out=scr[:, ts_lo:ts_hi],
                                in0=sfull[:, ts_lo:ts_hi],
                                scalar1=3.0e38, scalar2=None,
                                op0=MIN, op1=MIN,
                                accum_out=rowmin[:, nt:nt + 1])

            if row_mode == "foldx":
                # "fold" with HW-measured rebalancing knobs:
                #   CHAMFER_DVEDRAIN_PERIOD=k: every k-th n-tile, group 0 is
                #     drained PSUM->SBUF by DVE tensor_copy (HW: ~2us/group,
                #     faster than the erratum-slowed ACT ~3us) instead of ACT.
                #   CHAMFER_POOL_COLG=1: gpsimd does the col-min TT for the
                #     last m-group (frees ~1.1us/tile of DVE).
                #   CHAMFER_POOL_TREE=1: gpsimd does the 1024- and 512-wide
                #     row-tree levels (frees ~0.9us/tile of DVE).
                dve_period = int(os.environ.get(
                    "CHAMFER_DVEDRAIN_PERIOD", "0"))
                pool_colg = int(os.environ.get("CHAMFER_POOL_COLG", "0"))
                pool_tree = int(os.environ.get("CHAMFER_POOL_TREE", "0"))
                for rep in range(repeat):
                    for nt in range(nt_count):
                        dve_drain = dve_period and (
                            nt % dve_period == dve_period - 1)
                        lhsT = xTs[:kaug, nt * P:(nt + 1) * P]
                        sfull = wpool.tile([P, n], bf16, tag="s",
                                           name="sfull", bufs=3)
                        for g in range(ngroups):
                            ps = ppool.tile([P, m_group], fp32,
                                            tag="ps", name="ps")
                            for k in range(mm_per_g):
                                nc.tensor.matmul(
                                    ps[:, k * mm_free:(k + 1) * mm_free],
                                    lhsT,
                                    yTs[:kaug,
                                        g * m_group + k * mm_free:
                                        g * m_group + (k + 1) * mm_free],
                                    start=True, stop=True)
                            gsl = sfull[:, g * m_group:(g + 1) * m_group]
                            if dve_drain and g == 0:
                                nc.vector.tensor_copy(out=gsl, in_=ps)
                            else:
                                nc.scalar.copy(out=gsl, in_=ps)
                        # col-min accumulate
                        if nt == 0 and rep == 0:
                            nc.vector.tensor_copy(out=colacc, in_=sfull)
                        elif pool_colg:
                            csp = n - m_group
                            nc.vector.tensor_tensor(
                                out=colacc[:, :csp], in0=colacc[:, :csp],
                                in1=sfull[:, :csp], op=MIN)
                            nc.gpsimd.tensor_tensor(
                                out=colacc[:, csp:], in0=colacc[:, csp:],
                                in1=sfull[:, csp:], op=MIN)
                        else:
                            nc.vector.tensor_tensor(
                                out=colacc, in0=colacc, in1=sfull, op=MIN)
                        # row fold tree
                        u = wpool.tile([P, n // 2], bf16, tag="u",
                                       name="u", bufs=3)
                        nc.vector.tensor_tensor(
                            out=u, in0=sfull[:, :n // 2],
                            in1=sfull[:, n // 2:], op=MIN)
                        nc.vector.tensor_tensor(
                            out=u[:, :n // 4], in0=u[:, :n // 4],
                            in1=u[:, n // 4:n // 2], op=MIN)
                        w = n // 4
                        while w > 512:
                            eng = nc.gpsimd if pool_tree else nc.vector
                            eng.tensor_tensor(
                                out=u[:, :w // 2], in0=u[:, :w // 2],
                                in1=u[:, w // 2:w], op=MIN)
                            w //= 2
                        nc.vector.tensor_reduce(
                            out=rowmin[:, nt:nt + 1], in_=u[:, :w],
                            axis=mybir.AxisListType.X, op=MIN)

            if row_mode == "tsacc":
                # One n-wide s tile per n-tile: ONE wide col-min TT (2x) and
                # row mins via ONE tensor_scalar with min-accumulate (4x:
                # TensorScalarPtr supports 4x_2p; out=min(s,3e38)=s is a
                # throwaway wide write, accum_out = min over the free dim).
                for rep in range(repeat):
                    for nt in range(nt_count):
                        lhsT = xTs[:kaug, nt * P:(nt + 1) * P]
                        sfull = wpool.tile([P, n], bf16, tag="s",
                                           name="sfull", bufs=3)
                        for g in range(ngroups):
                            ps = ppool.tile([P, m_group], fp32,
                                            tag="ps", name="ps")
                            for k in range(mm_per_g):
                                nc.tensor.matmul(
                                    ps[:, k * mm_free:(k + 1) * mm_free],
                                    lhsT,
                                    yTs[:kaug,
                                        g * m_group + k * mm_free:
                                        g * m_group + (k + 1) * mm_free],
                                    start=True, stop=True)
                            nc.scalar.copy(
                                out=sfull[:, g * m_group:(g + 1) * m_group],
                                in_=ps)
                        if nt == 0 and rep == 0:
                            # TS both initializes colacc (out = min(s, 3e38)
                            # = s) and produces this tile's row min.
                            nc.vector.tensor_scalar(
                                out=colacc, in0=sfull, scalar1=3.0e38,
                                scalar2=None, op0=MIN, op1=MIN,
                                accum_out=rowmin[:, nt:nt + 1])
                        else:
                            nc.vector.tensor_tensor(
                                out=colacc, in0=colacc, in1=sfull, op=MIN)
                            scr = wpool.tile([P, n], bf16, tag="scr",
                                             name="scr", bufs=2)
                            nc.vector.tensor_scalar(
                                out=scr, in0=sfull, scalar1=3.0e38,
                                scalar2=None, op0=MIN, op1=MIN,
                                accum_out=rowmin[:, nt:nt + 1])

            if row_mode == "foldq":
                # "fold" with the row fold-trees of Q consecutive n-tiles
                # JOINED into one strided-AP instruction per tree level
                # (same element count at 2x, but 1/Q-th the DVE instruction
                # overhead, which HW shows is ~0.4-0.9us per op) and ONE
                # reduce producing Q rowmins at once.
                Q = int(os.environ.get("CHAMFER_Q", "2"))
                defer = int(os.environ.get("CHAMFER_DEFER", "4"))
                dve_period = int(os.environ.get(
                    "CHAMFER_DVEDRAIN_PERIOD", "0"))
                nq = nt_count // Q
                if defer:
                    # pending remnants: DEFER quads x Q tiles x 2048 each
                    pend = cpool.tile([P, defer * Q * 2048], bf16)
                    pv = pend.rearrange("p (d w) -> p d w", d=defer * Q)
                for rep in range(repeat):
                    for qi in range(nq):
                        squad = wpool.tile([P, Q * n], bf16, tag="sq",
                                           name="squad", bufs=2)
                        for t in range(Q):
                            nt = qi * Q + t
                            lhsT = xTs[:kaug, nt * P:(nt + 1) * P]
                            for g in range(ngroups):
                                ps = ppool.tile([P, m_group], fp32,
                                                tag="ps", name="ps")
                                for k in range(mm_per_g):
                                    nc.tensor.matmul(
                                        ps[:, k * mm_free:(k + 1) * mm_free],
                                        lhsT,
                                        yTs[:kaug,
                                            g * m_group + k * mm_free:
                                            g * m_group + (k + 1) * mm_free],
                                        start=True, stop=True)
                                gout = squad[:, t * n + g * m_group:
                                             t * n + (g + 1) * m_group]
                                if (dve_period and g == 0
                                        and nt % dve_period
                                        == dve_period - 1):
                                    nc.vector.tensor_copy(out=gout, in_=ps)
                                else:
                                    nc.scalar.copy(out=gout, in_=ps)
                            # col-min accumulate per sub-tile (fine-grained
                            # so it overlaps the next sub-tile's drains)
                            ssl = squad[:, t * n:(t + 1) * n]
                            if qi == 0 and t == 0 and rep == 0:
                                nc.vector.tensor_copy(out=colacc, in_=ssl)
                            else:
                                nc.vector.tensor_tensor(
                                    out=colacc, in0=colacc, in1=ssl, op=MIN)
                        # joint row fold tree over all Q sub-tiles
                        u = wpool.tile([P, Q * (n // 2)], bf16, tag="u",
                                       name="u", bufs=2)
                        sv = squad.rearrange("p (q w) -> p q w", q=Q)
                        uv = u.rearrange("p (q w) -> p q w", q=Q)
                        nc.vector.tensor_tensor(
                            out=uv[:, :, 0:n // 2], in0=sv[:, :, 0:n // 2],
                            in1=sv[:, :, n // 2:n], op=MIN)
                        if defer:
                            # L2 writes this quad's [Q, 2048] remnant into
                            # the pending buffer; every `defer` quads, the
                            # narrow levels + reduce run ONCE over all of
                            # them (fewer, wider DVE ops).
                            slot = (qi % defer) * Q
                            nc.vector.tensor_tensor(
                                out=pv[:, slot:slot + Q, :],
                                in0=uv[:, :, 0:2048],
                                in1=uv[:, :, 2048:4096], op=MIN)
                            if qi % defer == defer - 1:
                                # batched narrow levels: TT folding is 2x
                                # vs the 1x reduce, and at defer*Q tiles
                                # per op the fixed overhead amortizes, so
                                # fold well below 512 before reducing.
                                stop = int(os.environ.get(
                                    "CHAMFER_DEFER_STOP", "128"))
                                w = 2048
                                while w > stop:
                                    nc.vector.tensor_tensor(
                                        out=pv[:, :, 0:w // 2],
                                        in0=pv[:, :, 0:w // 2],
                                        in1=pv[:, :, w // 2:w], op=MIN)
                                    w //= 2
                                lo = (qi - defer + 1) * Q
                                nc.vector.tensor_reduce(
                                    out=rowmin[:, lo:lo + defer * Q],
                                    in_=pv[:, :, 0:w],
                                    axis=mybir.AxisListType.X, op=MIN)
                        else:
                            w = n // 2
                            while w > 512:
                                nc.vector.tensor_tensor(
                                    out=uv[:, :, 0:w // 2],
                                    in0=uv[:, :, 0:w // 2],
                                    in1=uv[:, :, w // 2:w], op=MIN)
                                w //= 2
                            nc.vector.tensor_reduce(
                                out=rowmin[:, qi * Q:(qi + 1) * Q],
                                in_=uv[:, :, 0:w],
                                axis=mybir.AxisListType.X, op=MIN)

            if row_mode == "fold":
                # One n-wide s tile per n-tile: ONE wide col-min TT, and row
                # mins via a fold tree of wide TT-mins + one small reduce.
                for rep in range(repeat):
                    for nt in range(nt_count):
                        lhsT = xTs[:kaug, nt * P:(nt + 1) * P]
                        sfull = wpool.tile([P, n], bf16, tag="s",
                                           name="sfull", bufs=3)
                        for g in range(ngroups):
                            ps = ppool.tile([P, m_group], fp32,
                                            tag="ps", name="ps")
                            for k in range(mm_per_g):
                                nc.tensor.matmul(
                                    ps[:, k * mm_free:(k + 1) * mm_free],
                                    lhsT,
                                    yTs[:kaug,
                                        g * m_group + k * mm_free:
                                        g * m_group + (k + 1) * mm_free],
                                    start=True, stop=True)
                            nc.scalar.copy(
                                out=sfull[:, g * m_group:(g + 1) * m_group],
                                in_=ps)
                        if nt == 0 and rep == 0:
                            nc.vector.tensor_copy(out=colacc, in_=sfull)
                        else:
                            nc.vector.tensor_tensor(
                                out=colacc, in0=colacc, in1=sfull, op=MIN)
                        # row fold tree
                        u = wpool.tile([P, n // 2], bf16, tag="u",
                                       name="u", bufs=3)
                        nc.vector.tensor_tensor(
                            out=u, in0=sfull[:, :n // 2],
                            in1=sfull[:, n // 2:], op=MIN)
                        w = n // 2
                        stop_w = int(os.environ.get("CHAMFER_TREE_STOP",
                                                    "512"))
                        while w > stop_w:
                            nc.vector.tensor_tensor(
                                out=u[:, :w // 2], in0=u[:, :w // 2],
                                in1=u[:, w // 2:w], op=MIN)
                            w //= 2
                        nc.vector.tensor_reduce(
                            out=rowmin[:, nt:nt + 1], in_=u[:, :w],
                            axis=mybir.AxisListType.X, op=MIN)

            for rep in range(
                    repeat if row_mode not in ("fold", "fold2", "tsacc",
                                               "tsacc2", "foldx", "foldq")
                    else 0):
              for nt in range(nt_count):
                lhsT = xTs[:kaug, nt * P:(nt + 1) * P]
                for g in range(ngroups):
                    ps = ppool.tile([P, m_group], fp32, tag="ps", name="ps")
                    for k in range(mm_per_g):
                        nc.tensor.matmul(
                            ps[:, k * mm_free:(k + 1) * mm_free],
                            lhsT,
                            yTs[:kaug, g * m_group + k * mm_free:
                                g * m_group + (k + 1) * mm_free],
                            start=True,
                            stop=True,
                        )
                    s = wpool.tile([P, m_group], bf16, name="s")
                    nc.scalar.copy(out=s, in_=ps)

                    # column-min accumulator (n folded into the 128 lanes)
                    csl = colacc[:, g * m_group:(g + 1) * m_group]
                    if nt == 0:
                        nc.vector.tensor_copy(out=csl, in_=s)
                    else:
                        nc.vector.tensor_tensor(out=csl, in0=csl, in1=s, op=MIN)

                    # row mins
                    if row_mode == "ttr2":
                        # like "ttr" but ping-pongs the elementwise-min
                        # accumulator to avoid in-place out/in1 aliasing
                        accs = [rowacc, rowacc2]
                        dst = accs[g % 2]
                        src = s if g == 0 else accs[1 - g % 2]
                        nc.vector.tensor_tensor_reduce(
                            out=dst,
                            in0=s,
                            in1=src,
                            scale=1.0,
                            scalar=3.0e38,
                            op0=MIN,
                            op1=MIN,
                            accum_out=rowmin[:, nt:nt + 1],
                        )
                    elif row_mode == "ttr":
                        # rowacc = min(rowacc, s) elementwise; accum_out gets
                        # min over the free dim of the updated rowacc. The
                        # last group's accum covers all m -> true row min.
                        nc.vector.tensor_tensor_reduce(
                            out=rowacc,
                            in0=s,
                            in1=(s if g == 0 else rowacc),
                            scale=1.0,
                            scalar=3.0e38,
                            op0=MIN,
                            op1=MIN,
                            accum_out=rowmin[:, nt:nt + 1],
                        )
                    else:
                        for k in range(mm_per_g):
                            ssl = s[:, k * mm_free:(k + 1) * mm_free]
                            if g == 0 and k == 0:
                                nc.vector.tensor_copy(out=rowacc_narrow, in_=ssl)
                            else:
                                nc.vector.tensor_tensor(
                                    out=rowacc_narrow, in0=rowacc_narrow,
                                    in1=ssl, op=MIN)
                        if g == ngroups - 1:
                            nc.vector.tensor_reduce(
                                out=rowmin[:, nt:nt + 1], in_=rowacc_narrow,
                                axis=mybir.AxisListType.X, op=MIN)

            # column-min finish: transpose each [128, 128] block of colacc on
            # PE, then min-reduce the (former partition) lanes on DVE.
            if col_tail == "host":
                nc.sync.dma_start(colout[:, :], final_colacc[:, :])
            elif not skip_tail:
                # batch transposes into wide bf16 PSUM tiles so the lane-min
                # runs as a few wide DVE reduces instead of nt_count small ones
                tpb = max(1, min(nt_count, (m_group * 2) // P))
                for t0 in range(0, nt_count, tpb):
                    cnt = min(tpb, nt_count - t0)
                    pt = ppool.tile([P, tpb, P], bf16, tag="ps", name="pt")
                    for i in range(cnt):
                        t = t0 + i
                        nc.tensor.transpose(
                            pt[:, i, :], final_colacc[:, t * P:(t + 1) * P], ident)
                    nc.vector.tensor_reduce(
                        out=colmin[:, t0:t0 + cnt], in_=pt[:, :cnt, :],
                        axis=mybir.AxisListType.X, op=MIN)
            else:
                nc.vector.tensor_copy(out=colmin, in_=rowmin)

            nc.sync.dma_start(out[:, :nt_count], rowmin[:, :])
            if row_mode == "tsacc2":
                nc.sync.dma_start(
                    out[:, nt_count:2 * nt_count], rowmin2[:, :])
            if col_tail != "host":
                nc.sync.dma_start(
                    out[:, (out_slots - 1) * nt_count:], colmin[:, :])

    nc.finalize()  # runs the Bacc compile passes (event sems, reg alloc, ...)
    return nc


def _prep_inputs(x, y, kaug=KAUG):
    """Build the augmented, transposed bf16 operands for each batch."""
    bf = ml_dtypes.bfloat16
    in_maps = []
    for b in range(x.shape[0]):
        xb = np.asarray(x[b], dtype=np.float32)
        yb = np.asarray(y[b], dtype=np.float32)
        n = xb.shape[0]
        x2 = np.sum(xb * xb, axis=-1)
        y2 = np.sum(yb * yb, axis=-1)
        x2_hi = x2.astype(bf)
        x2_lo = (x2 - x2_hi.astype(np.float32)).astype(bf)
        y2_hi = y2.astype(bf)
        y2_lo = (y2 - y2_hi.astype(np.float32)).astype(bf)
        ones = np.ones((1, n), dtype=bf)
        xT = np.concatenate(
            [xb.T.astype(bf), ones, ones, x2_hi[None], x2_lo[None]], axis=0)
        yT = np.concatenate(
            [(-2.0 * yb).T.astype(bf), y2_hi[None], y2_lo[None], ones, ones],
            axis=0)
        if kaug > KAUG:
            pad = np.zeros((kaug - KAUG, n), dtype=bf)
            xT = np.concatenate([xT, pad], axis=0)
            yT = np.concatenate([yT, pad], axis=0)
        in_maps.append({
            "xT": np.ascontiguousarray(xT),
            "yT": np.ascontiguousarray(yT),
        })
    return in_maps


def _postprocess(results, n=N):
    nt_count = n // P
    total = 0.0
    nb = len(results)
    for b in range(nb):
        o = np.asarray(results[b]["out"], dtype=np.float64)
        rowmin = o[:, :nt_count].T.reshape(-1)   # [n], index t*128+p
        if o.shape[1] == 3 * nt_count:
            # tsacc2: second slot holds DVE-drained group row partials
            rowmin = np.minimum(
                rowmin, o[:, nt_count:2 * nt_count].T.reshape(-1))
        if "colout" in results[b]:
            co = np.asarray(results[b]["colout"], dtype=np.float32)
            colmin = co.min(axis=0).astype(np.float64)
        else:
            colmin = o[:, -nt_count:].T.reshape(-1)
        total += np.sqrt(np.maximum(rowmin, 0.0)).sum()
        total += np.sqrt(np.maximum(colmin, 0.0)).sum()
    loss = total / nb / n
    return np.asarray(loss, dtype=np.float32)


def _get_runner(n_cores=B):
    """Build the Bass module once and return a reusable jitted runner.

    Modeled on concourse.bass2jax.run_bass_via_pjrt's multi-core branch, but
    keeps the jitted callable so repeated invocations don't re-lower."""
    key = ("runner", n_cores)
    if key in _CACHE:
        return _CACHE[key]

    import jax
    from jax.experimental.shard_map import shard_map
    from jax.sharding import Mesh, PartitionSpec
    from concourse import bass2jax, mybir

    nc = _build_nc(row_mode=os.environ.get("CHAMFER_ROW_MODE",
                                           DEFAULT_ROW_MODE),
                   col_tail=os.environ.get("CHAMFER_COL_TAIL", "device"))

    bass2jax.install_neuronx_cc_hook()
    assert nc.dbg_addr is None

    partition_name = (
        nc.partition_id_tensor.name if nc.partition_id_tensor else None)
    in_names, out_names, out_avals = [], [], []
    for alloc in nc.m.functions[0].allocations:
        if not isinstance(alloc, mybir.MemoryLocationSet):
            continue
        name = alloc.memorylocations[0].name
        if alloc.kind == "ExternalInput":
            if name != partition_name:
                in_names.append(name)
        elif alloc.kind == "ExternalOutput":
            out_names.append(name)
            out_avals.append(jax.core.ShapedArray(
                tuple(alloc.tensor_shape), mybir.dt.np(alloc.dtype)))
    n_params = len(in_names)
    n_outs = len(out_avals)
    all_in_names = list(in_names) + list(out_names)
    if partition_name is not None:
        all_in_names.append(partition_name)
    donate = tuple(range(n_params, n_params + n_outs))

    def _body(*args):
        operands = list(args)
        if partition_name is not None:
            operands.append(bass2jax.partition_id_tensor())
        outs = bass2jax._bass_exec_p.bind(
            *operands,
            out_avals=tuple(out_avals),
            in_names=tuple(all_in_names),
            out_names=tuple(out_names),
            lowering_input_output_aliases=(),
            sim_require_finite=True,
            sim_require_nnan=True,
            nc=nc,
        )
        return tuple(outs)

    devices = jax.devices()[:n_cores]
    mesh = Mesh(np.asarray(devices), ("core",))
    sharded = jax.jit(
        shard_map(
            _body, mesh=mesh,
            in_specs=(PartitionSpec("core"),) * (n_params + n_outs),
            out_specs=(PartitionSpec("core"),) * n_outs,
            check_rep=False,
        ),
        donate_argnums=donate,
        keep_unused=True,
    )

    def run(in_maps):
        per_core = [[np.asarray(m[nm]) for nm in in_names] for m in in_maps]
        concat_in = [
            np.concatenate([per_core[c][i] for c in range(n_cores)], axis=0)
            for i in range(n_params)
        ]
        concat_zeros = [
            np.zeros((n_cores * a.shape[0], *a.shape[1:]), a.dtype)
            for a in out_avals
        ]
        out_arrs = sharded(*concat_in, *concat_zeros)
        jax.block_until_ready(out_arrs)
        return [
            {nm: np.asarray(out_arrs[i]).reshape(
                n_cores, *out_avals[i].shape)[c]
             for i, nm in enumerate(out_names)}
            for c in range(n_cores)
        ]

    _CACHE[key] = run
    return run


def kernel(x, y):
    import time

    x = np.asarray(x)
    y = np.asarray(y)
    in_maps = _prep_inputs(x, y)
    run = _get_runner(n_cores=len(in_maps))
    # the device occasionally wedges transiently on a fresh NEFF's first
    # execution (NRT_EXEC_UNIT_UNRECOVERABLE); a retry reliably clears it
    last_err = None
    for attempt in range(4):
        try:
            results = run(in_maps)
            return _postprocess(results)
        except Exception as e:  # noqa: BLE001 - retry any runtime failure
            last_err = e
            time.sleep(2.0)
            try:
                import jax
                jax.clear_caches()
            except Exception:
                pass
            _CACHE.clear()  # rebuild runner; NEFF recompile is disk-cached
            run = _get_runner(n_cores=len(in_maps))
    raise last_err

